# revision 11
# baseline (speedup 1.0000x reference)
"""Two-layer GCN (PyG gcn_norm semantics) on 8 Trainium2 NeuronCores.

v2: fp8 DoubleRow identity-scatter (graph/data parallel, dst-sharded,
host-transported):

  - norm factorizes: norm(u->v) = dis[u]*dis[v], dis = (deg_in+1)^-1/2.
    Host pre-scales every edge message by its DESTINATION factor so the
    device epilogues are plain relu/copy (no per-window scale ops):
      L1 slot value = S1*dis2_v*T1[u]           (self: +S1*dis_v*b1)
      L2 slot value = S2*dis_v*T2[u]            (self: +S2*b2)
    with T1 = dis*(x@W1), T2 = z'@(W2/S1), z' = S1*dis*z. S1/S2 are
    power-of-two gains keeping fp8 e4m3 values in the normal range;
    1/S1 folds into the W2 weights, 1/S2 into the final output copy.

  - Streams are fp8 e4m3, aggregated with DoubleRow matmuls against a
    stacked identity: one matmul PSUM-accumulates TWO 128-edge blocks at
    0.5 cycles/row (4x bf16 throughput). Group k-depth is forced even.

  - Layer-1 tail: relu writes z' into 64-aligned window slots; XBAR DMA
    transposes window pairs SBUF->SBUF (no PE); W2 matmuls run with W2
    stationary and z'^T moving (512 node-cols per matmul), producing the
    T2 table feature-major for free host untransposition.

  - Three launches:
      NEFF-0: h1 = (dis*x) @ W1 per shard          -> [F1, nodes] bf16
      host:   gather+scale T1[src] into fp8 slot streams
      NEFF-A: L1 aggregation + relu + T2 = z'@W2'  -> [F2, nodes] bf16
      host:   gather+scale T2[src] (fp8)
      NEFF-B: L2 aggregation + 1/S2 copy           -> out bf16
"""

from dataclasses import dataclass

import numpy as np
import ml_dtypes

BF16 = ml_dtypes.bfloat16
E4M3 = ml_dtypes.float8_e4m3


@dataclass
class Config:
    N: int = 100000          # nodes
    F0: int = 128            # input features
    F1: int = 48             # hidden
    F2: int = 32             # out
    NC: int = 8              # cores
    PW: int = 128            # window (nodes per PSUM window)
    TB1: int = 10            # windows per group, layer 1 (TB1*F1 <= 512)
    TB2: int = 16            # windows per group, layer 2 (TB2*F2 <= 512)

    @property
    def NW(self):            # global windows (multiple of NC)
        nw = (self.N + self.PW - 1) // self.PW
        return ((nw + self.NC - 1) // self.NC) * self.NC

    @property
    def NPW(self):           # windows per core
        return self.NW // self.NC

    @property
    def SHARD_PAD(self):
        return self.NPW * self.PW


CFG = Config()


def _to_bf16(a):
    return np.asarray(a, dtype=np.float32).astype(BF16)


def _dedup_ldweights(nc):
    """Delete redundant InstLdweights: the PE array keeps its stationary
    matrix across matmuls, so a reload of the identical weights (and no
    semaphore wait/update riding on it) is dead work."""
    import concourse.mybir as mybir
    ndel = 0
    for fn in nc.m.functions:
        for blk in fn.blocks:
            keep, last_sig = [], None
            for inst in blk.instructions:
                if isinstance(inst, mybir.InstLdweights):
                    sig = inst.concise(deps=False)
                    if (sig == last_sig and not inst.has_wait()
                            and not inst.has_update()):
                        ndel += 1
                        continue
                    last_sig = sig
                elif (not isinstance(inst, mybir.InstMatmult)
                      and getattr(inst, "engine", None) == mybir.EngineType.PE
                      and inst.is_executable()):
                    last_sig = None
                keep.append(inst)
            blk.instructions = keep
    return ndel


def make_sched(cfg: Config, nb, TB, F, even_gn):
    """Partition consecutive windows into groups (DP-optimized sizes up
    to TB) with per-group even k-depth KG = even(max nb). Stream layout
    is k-major per group ([k][w][F]). Groups are listed (and processed)
    in descending-id = ascending-degree order."""
    NPW = cfg.NPW
    OH, CYC = 350.0, 0.83     # measured per-matmul overhead / PE ns-cycle
    step = 2 if even_gn else 1
    INF = float("inf")
    best = [INF] * (NPW + 1)
    best[0] = 0.0
    choice = [0] * (NPW + 1)
    for i in range(1, NPW + 1):
        mx = 0
        for gn in range(1, min(TB, i) + 1):
            mx = max(mx, int(nb[i - gn]))
            if even_gn and gn % 2:
                continue
            KG = mx + (mx % 2)
            cost = (KG // 2) * (OH + gn * F * 0.5 * CYC)
            if best[i - gn] + cost < best[i]:
                best[i] = best[i - gn] + cost
                choice[i] = gn
    bounds = []
    i = NPW
    while i > 0:
        gn = choice[i]
        bounds.append((i - gn, gn))
        i -= gn
    # bounds already descending by window id (ascending degree)
    groups = []
    maxnb = int(max(nb))
    lut = np.full((NPW, maxnb), -1, dtype=np.int64)
    blk = 0
    for lo, gn in bounds:
        wins = list(range(lo, lo + gn))     # ascending ids
        KG = max(int(nb[w]) for w in wins)
        KG += KG % 2                        # force even for DoubleRow
        for wi, w in enumerate(wins):
            for k in range(int(nb[w])):
                lut[w, k] = blk + k * gn + wi
        groups.append({"wins": wins, "w0": lo, "gn": gn, "KG": KG,
                       "start": blk})
        blk += gn * KG
    return {"groups": groups, "lut": lut, "B": blk, "maxKG": max(
        g["KG"] for g in groups)}


def preprocess(cfg: Config, edge_index):
    N, NC, PW, NPW = cfg.N, cfg.NC, cfg.PW, cfg.NPW
    NW = cfg.NW

    src = np.asarray(edge_index[0], dtype=np.int64)
    dst = np.asarray(edge_index[1], dtype=np.int64)
    E = src.shape[0]

    indeg = np.bincount(dst, minlength=N)
    degp1 = indeg.astype(np.float64) + 1.0
    dis = (degp1 ** -0.5).astype(np.float32)
    dis2 = (degp1 ** -1.0).astype(np.float32)

    perm = np.argsort(-indeg, kind="stable")       # rank -> orig node
    rank = np.empty(N, dtype=np.int64)
    rank[perm] = np.arange(N)

    indeg_sorted = indeg[perm]                     # descending
    win_max = np.zeros(NW, dtype=np.int64)
    nwin_real = (N + PW - 1) // PW
    win_max[:nwin_real] = indeg_sorted[::PW][:nwin_real]
    nb = 1 + win_max.reshape(NPW, NC).max(axis=1)  # shared schedule [NPW]

    # node at (core c, local window g, pos p) = perm[(g*NC + c)*PW + p]
    node_of = []
    slots_all = np.full(NW * PW, -1, dtype=np.int64)
    slots_all[:N] = perm
    grid = slots_all.reshape(NPW, NC, PW)          # [g, c, p]
    for c in range(NC):
        node_of.append(np.ascontiguousarray(grid[:, c, :]).reshape(-1))

    # per-edge position: k-th in-edge (k starting at 1; 0 = self)
    rd = rank[dst]
    order_e = np.argsort(rd, kind="stable")
    src_s = src[order_e]
    rd_s = rd[order_e]
    cum = np.concatenate([[0], np.cumsum(indeg_sorted)])
    k_e = np.arange(E) - cum[rd_s] + 1             # 1..indeg
    wg = rd_s // PW
    p_e = rd_s % PW
    g_e = wg // NC                                 # local window
    c_e = wg % NC                                  # core
    dis_r = dis[perm]                              # by rank
    dis2_r = dis2[perm]

    meta = {"nb": nb, "node_of": node_of, "dis": dis, "dis2": dis2,
            "perm": perm}

    for layer, TB in ((1, cfg.TB1), (2, cfg.TB2)):
        sch = make_sched(cfg, nb, TB, cfg.F1 if layer == 1 else cfg.F2,
                         even_gn=(layer == 1))
        lut, B = sch["lut"], sch["B"]
        slot_e = lut[g_e, k_e] * PW + p_e
        dsc_e = (dis2_r if layer == 1 else dis_r)[rd_s]
        self_blocks = lut[:, 0]                    # [NPW]
        self_slots = (self_blocks[:, None] * PW
                      + np.arange(PW)[None, :]).reshape(-1)
        sid_c, dsc_c = [], []
        for c in range(NC):
            sid = np.full(B * PW, -1, dtype=np.int64)
            dsc = np.zeros(B * PW, dtype=np.float32)
            m = c_e == c
            sid[slot_e[m]] = src_s[m]
            dsc[slot_e[m]] = dsc_e[m]
            nod = node_of[c]
            valid = nod >= 0
            sv = np.zeros(cfg.SHARD_PAD, dtype=np.float32)
            sv[valid] = (dis2 if layer == 1 else dis)[nod[valid]]
            sid[self_slots] = nod
            dsc[self_slots] = sv
            sid_c.append(sid)
            dsc_c.append(dsc)
        sch["sid"] = sid_c
        sch["dsc"] = dsc_c
        sch["self_slots"] = self_slots
        meta[f"sched{layer}"] = sch
    return meta


def pow2_gain(mx, target=240.0):
    if mx <= 0:
        return 1.0
    return float(2.0 ** np.floor(np.log2(target / mx)))


def gather_stream(cfg: Config, sch, c, table, F, S, self_extra):
    """table [N, F] f32 -> [128, B*F] e4m3 slot stream for core c.
    Slot value = S * dsc[slot] * table[sid[slot]]; self_extra [SHARD_PAD, F]
    (S * per-node bias term) is added onto the self-loop slots."""
    sid, dsc = sch["sid"][c], sch["dsc"][c]
    B = sid.shape[0] // cfg.PW
    m = np.zeros((sid.shape[0], F), dtype=np.float32)
    valid = sid >= 0
    m[valid] = table[sid[valid]] * (dsc[valid] * S)[:, None]
    if self_extra is not None:
        m[sch["self_slots"]] += self_extra
    m = m.astype(E4M3)
    # slot s = b*128 + p  ->  [p, b, f]
    m = np.ascontiguousarray(m.reshape(B, cfg.PW, F).transpose(1, 0, 2))
    return m.reshape(cfg.PW, B * F)


def unpack_feature_major(cfg: Config, tab, rows, node_of):
    """rows [F, SHARD_PAD] device output -> scatter into full [N, F]
    table by orig node id (cores own disjoint node sets)."""
    a = np.asarray(rows, dtype=np.float32).T       # [SHARD_PAD, F]
    valid = node_of >= 0
    tab[node_of[valid]] = a[valid]


def build_dense(cfg: Config):
    """NEFF-0: h1 = xT.T @ W1 per shard (xT pre-scaled by dis on host)."""
    import concourse.bacc as bacc
    import concourse.mybir as mybir
    from concourse import tile

    dt = mybir.dt
    AF = mybir.ActivationFunctionType
    NPW, PW, F0, F1 = cfg.NPW, cfg.PW, cfg.F0, cfg.F1

    nc = bacc.Bacc("TRN2", target_bir_lowering=False, debug=False,
                   num_devices=cfg.NC)
    nc.move_matmul_waits_to_ldweights = lambda: None
    xT = nc.dram_tensor("xT", [F0, cfg.SHARD_PAD], dt.bfloat16,
                        kind="ExternalInput")
    W1t = nc.dram_tensor("W1t", [F0, F1], dt.bfloat16, kind="ExternalInput")
    h1 = nc.dram_tensor("h1", [F1, cfg.SHARD_PAD], dt.bfloat16,
                        kind="ExternalOutput")

    GW = 4    # windows per matmul (512 moving cols)
    with tile.TileContext(nc) as tc:
        with (
            tc.tile_pool(name="const", bufs=1) as constp,
            tc.tile_pool(name="ps", bufs=4, space="PSUM") as psp,
        ):
            w1s = constp.tile([F0, F1], dt.bfloat16)
            nc.sync.dma_start(w1s[:, :], W1t[:, :])
            xt = constp.tile([128, cfg.SHARD_PAD], dt.bfloat16)
            # small leading chunk (on scalar, parallel to W1 on sync) so
            # the first matmul starts early
            segs = [(0, 512)]
            a = 512
            while a < cfg.SHARD_PAD:
                b = min(cfg.SHARD_PAD, a + 1024)
                segs.append((a, b - a))
                a = b
            for i, (a, n) in enumerate(segs):
                eng = (nc.scalar, nc.sync)[i % 2]
                eng.dma_start(xt[:, a:a + n], xT[:, a:a + n])
            h_full = constp.tile([F1, cfg.SHARD_PAD], dt.bfloat16)
            wrote = 0
            for g0 in range(0, NPW, GW):
                gn = min(GW, NPW - g0)
                ps = psp.tile([F1, GW * PW], dt.float32, tag="ps")
                nc.tensor.matmul(out=ps[:, :gn * PW], lhsT=w1s[:, :],
                                 rhs=xt[:, g0 * PW:(g0 + gn) * PW],
                                 start=True, stop=True)
                if (g0 // GW) % 2 == 0:
                    nc.scalar.activation(
                        h_full[:, g0 * PW:(g0 + gn) * PW],
                        ps[:, :gn * PW], AF.Copy)
                else:
                    nc.vector.tensor_copy(
                        h_full[:, g0 * PW:(g0 + gn) * PW],
                        ps[:, :gn * PW])
                done = g0 + gn
                if done - wrote >= 16 or done == NPW:
                    eng = (nc.sync, nc.scalar)[(wrote // 16) % 2]
                    eng.dma_start(h1[:, wrote * PW:done * PW],
                                  h_full[:, wrote * PW:done * PW])
                    wrote = done
    _dedup_ldweights(nc)
    nc.compile()
    return nc


def build_edge(cfg: Config, sch, layer):
    """NEFF-A (layer=1): fp8 DoubleRow aggregation + relu -> z';
         XBAR pair transposes; T2 = z'^T.T @ W2'   -> h2 [F2, nodes] bf16
       NEFF-B (layer=2): fp8 DoubleRow aggregation + (1/S2) copy
                                                    -> out [128, NPW*F2]
    """
    import concourse.bacc as bacc
    import concourse.mybir as mybir
    from concourse import tile
    from concourse.masks import make_identity

    dt = mybir.dt
    AF = mybir.ActivationFunctionType
    DR = mybir.MatmulPerfMode.DoubleRow
    NPW, PW = cfg.NPW, cfg.PW
    F1, F2 = cfg.F1, cfg.F2
    FM = F1 if layer == 1 else F2
    groups, B, maxKG = sch["groups"], sch["B"], sch["maxKG"]
    TB = cfg.TB1 if layer == 1 else cfg.TB2

    nc = bacc.Bacc("TRN2", target_bir_lowering=False, debug=False,
                   num_devices=cfg.NC)
    nc.move_matmul_waits_to_ldweights = lambda: None

    msgs = nc.dram_tensor("msgs", [128, B * FM], dt.float8e4,
                          kind="ExternalInput")
    id2 = nc.dram_tensor("id2", [128, 2 * 128], dt.float8e4,
                         kind="ExternalInput")   # stacked identity
    if layer == 1:
        W2t = nc.dram_tensor("W2t", [128, F2], dt.bfloat16,
                             kind="ExternalInput")  # W2/S1 at rows 0:48,64:112
        h2 = nc.dram_tensor("h2", [F2, cfg.SHARD_PAD], dt.bfloat16,
                            kind="ExternalOutput")
    else:
        inv = nc.dram_tensor("inv", [PW, 1], dt.float32,
                             kind="ExternalInput")  # 1/S2
        out = nc.dram_tensor("out", [128, NPW * F2], dt.bfloat16,
                             kind="ExternalOutput")

    dve_groups = {0, 2, 4} if layer == 1 else {0, 2}
    with tile.TileContext(nc) as tc:
        with (
            tc.tile_pool(name="const", bufs=1) as constp,
            tc.tile_pool(name="zr", bufs=2) as zrp,
            tc.tile_pool(name="ps", bufs=3, space="PSUM") as psp,
            tc.tile_pool(name="psw", bufs=2, space="PSUM") as pswp,
        ):
            # The whole fp8 stream fits in SBUF: fetch EVERY group upfront
            # into exact-size tiles (no buffer reuse, so the sync queue
            # streams back-to-back with zero waits).
            gtiles = []
            for g in groups:
                n = g["KG"] * g["gn"] * FM
                gt = constp.tile([128, n], dt.float8e4)
                s0 = g["start"] * FM
                nc.sync.dma_start(gt[:, :], msgs[:, s0:s0 + n])
                gtiles.append(gt)

            ident2 = constp.tile([128, 2, 128], dt.float8e4)
            nc.scalar.dma_start(ident2[:, :, :], id2[:, :])
            if layer == 1:
                w2s = constp.tile([128, F2], dt.bfloat16)
                nc.scalar.dma_start(w2s[:, :], W2t[:, :])
                z_all = constp.tile([128, NPW * 64], dt.bfloat16)
                zT = constp.tile([128, (NPW // 2) * 128], dt.bfloat16)
                t2_all = constp.tile([F2, cfg.SHARD_PAD], dt.bfloat16)
                t2q = t2_all[:, :].rearrange("a (j rq) -> a j rq", rq=256)
            else:
                invs = constp.tile([PW, 1], dt.float32)
                nc.scalar.dma_start(invs[:, :], inv[:, :])
                o_full = constp.tile([128, NPW * F2], dt.bfloat16)

            def emit_w2(p_lo, p_hi):
                """T2 = z'^T.T @ W2' over transposed pairs [p_lo, p_hi):
                evens then odds series, 512-col chunks, PSUM->t2_all on
                vector, then flush exactly these windows to DRAM."""
                ccols = (p_hi - p_lo) * 128
                for half, r0 in ((0, 0), (1, 64)):
                    for c0 in range(0, ccols, 512):
                        cw = min(512, ccols - c0)
                        ps2 = pswp.tile([F2, 512], dt.float32, tag="ps2")
                        nc.tensor.matmul(
                            out=ps2[:, :cw],
                            lhsT=w2s[r0:r0 + F1, :],
                            rhs=zT[r0:r0 + F1,
                                   p_lo * 128 + c0:p_lo * 128 + c0 + cw],
                            start=True, stop=True)
                        npc = cw // 128
                        j0 = p_lo + c0 // 128
                        dv = t2q[:, j0:j0 + npc,
                                 half * 128:half * 128 + 128]
                        sv = ps2[:, :cw].rearrange("a (j q) -> a j q", q=128)
                        nc.vector.tensor_copy(dv, sv)
                nc.gpsimd.dma_start(h2[:, p_lo * 256:p_hi * 256],
                                    t2_all[:, p_lo * 256:p_hi * 256])

            wrote = [NPW]   # layer-2 flush high-water (window ids descend)
            pend = [None, None]   # transposed-but-not-W2'd pair range
            ready = [None]        # pairs transposed through groups <= gi-1
            for gi, g in enumerate(groups):
                gn, KG, w0 = g["gn"], g["KG"], g["w0"]
                gt = gtiles[gi]
                cols = gn * FM
                gv = gt[:, :].rearrange("p (k c) -> p k c", c=cols)
                if gi in dve_groups:
                    # DVE strided-k reduce keeps the PE free
                    acc = zrp.tile([128, TB * FM], dt.float32, tag="zr")
                    nc.vector.tensor_reduce(
                        acc[:, :cols], gv.rearrange("p k c -> p c k"),
                        mybir.AxisListType.X, mybir.AluOpType.add)
                else:
                    acc = psp.tile([128, TB * FM], dt.float32, tag="ps")
                    for k in range(0, KG, 2):
                        nc.tensor.matmul(out=acc[:, :cols],
                                         lhsT=ident2[:, :, :],
                                         rhs=gv[:, k:k + 2, :],
                                         start=(k == 0), stop=(k == KG - 2),
                                         perf_mode=DR)
                if layer == 1:
                    # relu -> z' into 64-aligned window slots (one ACT)
                    zv = z_all[:, w0 * 64:(w0 + gn) * 64].rearrange(
                        "p (w f) -> p w f", f=64)[:, :, 0:F1]
                    pv = acc[:, :cols].rearrange("p (w f) -> p w f", f=F1)
                    nc.scalar.activation(zv, pv, AF.Relu)
                    # XBAR pair transposes: [128, gn*64] -> gn/2 slabs
                    p0 = w0 // 2
                    npair = gn // 2
                    tv = zT[:, p0 * 128:(p0 + npair) * 128].rearrange(
                        "p (j q) -> p j q", q=128)
                    nc.scalar.dma_start(tv,
                                        z_all[:, w0 * 64:(w0 + gn) * 64],
                                        transpose=True)
                    # W2 lags one group so its XBAR wait never stalls the
                    # in-order PE queue
                    if pend[1] is None:
                        pend[1] = p0 + npair
                    pend[0] = p0
                    ready[0] = pend[0] + npair   # exclude current group
                    if pend[1] - ready[0] >= 8:
                        emit_w2(ready[0], pend[1])
                        pend[1] = ready[0]
                else:
                    ov = o_full[:, w0 * F2:(w0 + gn) * F2]
                    if gi in dve_groups:
                        nc.vector.tensor_scalar_mul(ov, acc[:, :cols],
                                                    invs[:, :])
                    elif gi % 2 == 0:
                        nc.vector.tensor_scalar_mul(ov, acc[:, :cols],
                                                    invs[:, :])
                    else:
                        nc.scalar.activation(ov, acc[:, :cols], AF.Copy,
                                             scale=invs[:, :])
                    hi = wrote[0]
                    if hi - w0 >= 24 or gi == len(groups) - 1:
                        nc.scalar.dma_start(out[:, w0 * F2:hi * F2],
                                            o_full[:, w0 * F2:hi * F2])
                        wrote[0] = w0
            if layer == 1 and pend[1] is not None and pend[1] > pend[0]:
                emit_w2(pend[0], pend[1])
    _dedup_ldweights(nc)
    nc.compile()
    return nc


EXEC_LOG = []  # (exec_time_ns, trace_path) per launch when BASS_TRACE=1


def run_spmd(cfg: Config, nc, in_maps):
    from concourse.bass_utils import run_bass_kernel_spmd
    res = run_bass_kernel_spmd(nc, in_maps=in_maps,
                               core_ids=list(range(cfg.NC)))
    trace_path = None
    if res.instructions_and_trace is not None:
        trace_path = res.instructions_and_trace[1]
    EXEC_LOG.append((res.exec_time_ns, trace_path))
    return res.results


def kernel(x, edge_index, W1, b1, W2, b2):
    cfg = CFG
    N, NC, PW, NPW = cfg.N, cfg.NC, cfg.PW, cfg.NPW
    meta = preprocess(cfg, edge_index)
    dis, dis2 = meta["dis"], meta["dis2"]
    sqd = 1.0 / dis

    x = np.asarray(x, dtype=np.float32)
    xs = x * dis[:, None]
    b1 = np.asarray(b1, dtype=np.float32).reshape(1, cfg.F1)
    b2 = np.asarray(b2, dtype=np.float32).reshape(1, cfg.F2)

    in0 = []
    for c in range(NC):
        nod = meta["node_of"][c]
        valid = nod >= 0
        xc = np.zeros((cfg.SHARD_PAD, cfg.F0), dtype=np.float32)
        xc[valid] = xs[nod[valid]]
        xT = np.ascontiguousarray(xc.T).astype(BF16)
        in0.append({"xT": xT, "W1t": _to_bf16(W1)})

    nc0 = build_dense(cfg)
    res0 = run_spmd(cfg, nc0, in0)
    T1 = np.zeros((N, cfg.F1), dtype=np.float32)
    for c in range(NC):
        unpack_feature_major(cfg, T1, res0[c]["h1"], meta["node_of"][c])

    # S1: max |stream value| = max(dis2_v*|T1[u]|, dis2_v*|T1[v]+sqd_v*b1|)
    sch1 = meta["sched1"]
    rmax1 = np.abs(T1).max(axis=1)
    selfv1 = T1 + sqd[:, None] * b1
    mx = 0.0
    for c in range(NC):
        sid, dsc = sch1["sid"][c], sch1["dsc"][c]
        v = sid >= 0
        m = float((np.abs(rmax1[sid[v]]) * dsc[v]).max())
        mx = max(mx, m)
    mx = max(mx, float((dis2[:, None] * np.abs(selfv1)).max()))
    S1 = pow2_gain(mx)

    ncA = build_edge(cfg, sch1, layer=1)
    eye = np.eye(128, dtype=np.float32)
    id2 = np.concatenate([eye, eye], axis=1).astype(E4M3)
    w2dup = np.zeros((128, cfg.F2), dtype=np.float32)
    w2v = np.asarray(W2, dtype=np.float32) / S1
    w2dup[0:cfg.F1] = w2v
    w2dup[64:64 + cfg.F1] = w2v
    inA = []
    for c in range(NC):
        nod = meta["node_of"][c]
        valid = nod >= 0
        ext = np.zeros((cfg.SHARD_PAD, cfg.F1), dtype=np.float32)
        ext[valid] = (S1 * dis[nod[valid], None]) * b1
        inA.append({"msgs": gather_stream(cfg, sch1, c, T1, cfg.F1, S1, ext),
                    "W2t": _to_bf16(w2dup), "id2": id2})
    resA = run_spmd(cfg, ncA, inA)
    T2 = np.zeros((N, cfg.F2), dtype=np.float32)
    for c in range(NC):
        unpack_feature_major(cfg, T2, resA[c]["h2"], meta["node_of"][c])

    sch2 = meta["sched2"]
    rmax2 = np.abs(T2).max(axis=1)
    selfv2 = dis[:, None] * T2 + b2
    mx = 0.0
    for c in range(NC):
        sid, dsc = sch2["sid"][c], sch2["dsc"][c]
        v = sid >= 0
        m = float((np.abs(rmax2[sid[v]]) * dsc[v]).max())
        mx = max(mx, m)
    mx = max(mx, float(np.abs(selfv2).max()))
    S2 = pow2_gain(mx)

    ncB = build_edge(cfg, sch2, layer=2)
    inB = []
    invv = np.full((PW, 1), 1.0 / S2, dtype=np.float32)
    for c in range(NC):
        nod = meta["node_of"][c]
        valid = nod >= 0
        # self slot extra: dsc already carries dis_v; slot = S2*dis_v*T2[v]
        # + S2*b2  (dis*sqd = 1)
        ext = np.zeros((cfg.SHARD_PAD, cfg.F2), dtype=np.float32)
        ext[valid] = S2 * b2
        inB.append({"msgs": gather_stream(cfg, sch2, c, T2, cfg.F2, S2, ext),
                    "inv": invv, "id2": id2})
    resB = run_spmd(cfg, ncB, inB)

    out = np.zeros((N, cfg.F2), dtype=np.float32)
    for c in range(NC):
        rows = np.asarray(resB[c]["out"]).astype(np.float32)
        a = rows.reshape(cfg.PW, NPW, cfg.F2).transpose(1, 0, 2).reshape(
            -1, cfg.F2)
        nod = meta["node_of"][c]
        valid = nod >= 0
        out[nod[valid]] = a[valid]
    return out


# revision 12
# speedup vs baseline: 1.1749x; 1.1749x over previous
"""Two-layer GCN (PyG gcn_norm semantics) on 8 Trainium2 NeuronCores.

v2: fp8 DoubleRow identity-scatter (graph/data parallel, dst-sharded,
host-transported):

  - norm factorizes: norm(u->v) = dis[u]*dis[v], dis = (deg_in+1)^-1/2.
    Host pre-scales every edge message by its DESTINATION factor so the
    device epilogues are plain relu/copy (no per-window scale ops):
      L1 slot value = S1*dis2_v*T1[u]           (self: +S1*dis_v*b1)
      L2 slot value = S2*dis_v*T2[u]            (self: +S2*b2)
    with T1 = dis*(x@W1), T2 = z'@(W2/S1), z' = S1*dis*z. S1/S2 are
    power-of-two gains keeping fp8 e4m3 values in the normal range;
    1/S1 folds into the W2 weights, 1/S2 into the final output copy.

  - Streams are fp8 e4m3, aggregated with DoubleRow matmuls against a
    stacked identity: one matmul PSUM-accumulates TWO 128-edge blocks at
    0.5 cycles/row (4x bf16 throughput). Group k-depth is forced even.

  - Layer-1 tail: relu writes z' into 64-aligned window slots; XBAR DMA
    transposes window pairs SBUF->SBUF (no PE); W2 matmuls run with W2
    stationary and z'^T moving (512 node-cols per matmul), producing the
    T2 table feature-major for free host untransposition.

  - Three launches:
      NEFF-0: h1 = (dis*x) @ W1 per shard          -> [F1, nodes] bf16
      host:   gather+scale T1[src] into fp8 slot streams
      NEFF-A: L1 aggregation + relu + T2 = z'@W2'  -> [F2, nodes] bf16
      host:   gather+scale T2[src] (fp8)
      NEFF-B: L2 aggregation + 1/S2 copy           -> out bf16
"""

from dataclasses import dataclass

import numpy as np
import ml_dtypes

BF16 = ml_dtypes.bfloat16
E4M3 = ml_dtypes.float8_e4m3


@dataclass
class Config:
    N: int = 100000          # nodes
    F0: int = 128            # input features
    F1: int = 48             # hidden
    F2: int = 32             # out
    NC: int = 8              # cores
    PW: int = 128            # window (nodes per PSUM window)
    TB1: int = 10            # windows per group, layer 1 (TB1*F1 <= 512)
    TB2: int = 16            # windows per group, layer 2 (TB2*F2 <= 512)

    @property
    def NW(self):            # global windows (multiple of NC)
        nw = (self.N + self.PW - 1) // self.PW
        return ((nw + self.NC - 1) // self.NC) * self.NC

    @property
    def NPW(self):           # windows per core
        return self.NW // self.NC

    @property
    def SHARD_PAD(self):
        return self.NPW * self.PW


CFG = Config()


def _to_bf16(a):
    return np.asarray(a, dtype=np.float32).astype(BF16)


def _dedup_ldweights(nc):
    """Delete redundant InstLdweights: the PE array keeps its stationary
    matrix across matmuls, so a reload of the identical weights (and no
    semaphore wait/update riding on it) is dead work."""
    import concourse.mybir as mybir
    ndel = 0
    for fn in nc.m.functions:
        for blk in fn.blocks:
            keep, last_sig = [], None
            for inst in blk.instructions:
                if isinstance(inst, mybir.InstLdweights):
                    sig = inst.concise(deps=False)
                    if (sig == last_sig and not inst.has_wait()
                            and not inst.has_update()):
                        ndel += 1
                        continue
                    last_sig = sig
                elif (not isinstance(inst, mybir.InstMatmult)
                      and getattr(inst, "engine", None) == mybir.EngineType.PE
                      and inst.is_executable()):
                    last_sig = None
                keep.append(inst)
            blk.instructions = keep
    return ndel


def make_sched(cfg: Config, nb, TB, F, even_gn):
    """Partition consecutive windows into groups (DP-optimized sizes up
    to TB) with per-group even k-depth KG = even(max nb). Stream layout
    is k-major per group ([k][w][F]). Groups are listed (and processed)
    in descending-id = ascending-degree order."""
    NPW = cfg.NPW
    OH, CYC = 350.0, 0.83     # measured per-matmul overhead / PE ns-cycle
    step = 2 if even_gn else 1
    INF = float("inf")
    best = [INF] * (NPW + 1)
    best[0] = 0.0
    choice = [0] * (NPW + 1)
    for i in range(1, NPW + 1):
        mx = 0
        for gn in range(1, min(TB, i) + 1):
            mx = max(mx, int(nb[i - gn]))
            if even_gn and gn % 2:
                continue
            KG = mx + (mx % 2)
            cost = (KG // 2) * (OH + gn * F * 0.5 * CYC)
            if best[i - gn] + cost < best[i]:
                best[i] = best[i - gn] + cost
                choice[i] = gn
    bounds = []
    i = NPW
    while i > 0:
        gn = choice[i]
        bounds.append((i - gn, gn))
        i -= gn
    # bounds already descending by window id (ascending degree)
    groups = []
    maxnb = int(max(nb))
    lut = np.full((NPW, maxnb), -1, dtype=np.int64)
    blk = 0
    for lo, gn in bounds:
        wins = list(range(lo, lo + gn))     # ascending ids
        KG = max(int(nb[w]) for w in wins)
        KG += KG % 2                        # force even for DoubleRow
        for wi, w in enumerate(wins):
            for k in range(int(nb[w])):
                lut[w, k] = blk + k * gn + wi
        groups.append({"wins": wins, "w0": lo, "gn": gn, "KG": KG,
                       "start": blk})
        blk += gn * KG
    return {"groups": groups, "lut": lut, "B": blk, "maxKG": max(
        g["KG"] for g in groups)}


def preprocess(cfg: Config, edge_index):
    N, NC, PW, NPW = cfg.N, cfg.NC, cfg.PW, cfg.NPW
    NW = cfg.NW

    src = np.asarray(edge_index[0], dtype=np.int64)
    dst = np.asarray(edge_index[1], dtype=np.int64)
    E = src.shape[0]

    indeg = np.bincount(dst, minlength=N)
    degp1 = indeg.astype(np.float64) + 1.0
    dis = (degp1 ** -0.5).astype(np.float32)
    dis2 = (degp1 ** -1.0).astype(np.float32)

    perm = np.argsort(-indeg, kind="stable")       # rank -> orig node
    rank = np.empty(N, dtype=np.int64)
    rank[perm] = np.arange(N)

    indeg_sorted = indeg[perm]                     # descending
    win_max = np.zeros(NW, dtype=np.int64)
    nwin_real = (N + PW - 1) // PW
    win_max[:nwin_real] = indeg_sorted[::PW][:nwin_real]
    nb = 1 + win_max.reshape(NPW, NC).max(axis=1)  # shared schedule [NPW]

    # node at (core c, local window g, pos p) = perm[(g*NC + c)*PW + p]
    node_of = []
    slots_all = np.full(NW * PW, -1, dtype=np.int64)
    slots_all[:N] = perm
    grid = slots_all.reshape(NPW, NC, PW)          # [g, c, p]
    for c in range(NC):
        node_of.append(np.ascontiguousarray(grid[:, c, :]).reshape(-1))

    # per-edge position: k-th in-edge (k starting at 1; 0 = self)
    rd = rank[dst]
    order_e = np.argsort(rd, kind="stable")
    src_s = src[order_e]
    rd_s = rd[order_e]
    cum = np.concatenate([[0], np.cumsum(indeg_sorted)])
    k_e = np.arange(E) - cum[rd_s] + 1             # 1..indeg
    wg = rd_s // PW
    p_e = rd_s % PW
    g_e = wg // NC                                 # local window
    c_e = wg % NC                                  # core
    dis_r = dis[perm]                              # by rank
    dis2_r = dis2[perm]

    meta = {"nb": nb, "node_of": node_of, "dis": dis, "dis2": dis2,
            "perm": perm}

    for layer, TB in ((1, cfg.TB1), (2, cfg.TB2)):
        sch = make_sched(cfg, nb, TB, cfg.F1 if layer == 1 else cfg.F2,
                         even_gn=(layer == 1))
        lut, B = sch["lut"], sch["B"]
        slot_e = lut[g_e, k_e] * PW + p_e
        dsc_e = (dis2_r if layer == 1 else dis_r)[rd_s]
        self_blocks = lut[:, 0]                    # [NPW]
        self_slots = (self_blocks[:, None] * PW
                      + np.arange(PW)[None, :]).reshape(-1)
        sid_c, dsc_c = [], []
        for c in range(NC):
            sid = np.full(B * PW, -1, dtype=np.int64)
            dsc = np.zeros(B * PW, dtype=np.float32)
            m = c_e == c
            sid[slot_e[m]] = src_s[m]
            dsc[slot_e[m]] = dsc_e[m]
            nod = node_of[c]
            valid = nod >= 0
            sv = np.zeros(cfg.SHARD_PAD, dtype=np.float32)
            sv[valid] = (dis2 if layer == 1 else dis)[nod[valid]]
            sid[self_slots] = nod
            dsc[self_slots] = sv
            sid_c.append(sid)
            dsc_c.append(dsc)
        sch["sid"] = sid_c
        sch["dsc"] = dsc_c
        sch["self_slots"] = self_slots
        meta[f"sched{layer}"] = sch
    return meta


def pow2_gain(mx, target=240.0):
    if mx <= 0:
        return 1.0
    return float(2.0 ** np.floor(np.log2(target / mx)))


def gather_stream(cfg: Config, sch, c, table, F, S, self_extra):
    """table [N, F] f32 -> [128, B*F] e4m3 slot stream for core c.
    Slot value = S * dsc[slot] * table[sid[slot]]; self_extra [SHARD_PAD, F]
    (S * per-node bias term) is added onto the self-loop slots."""
    sid, dsc = sch["sid"][c], sch["dsc"][c]
    B = sid.shape[0] // cfg.PW
    m = np.zeros((sid.shape[0], F), dtype=np.float32)
    valid = sid >= 0
    m[valid] = table[sid[valid]] * (dsc[valid] * S)[:, None]
    if self_extra is not None:
        m[sch["self_slots"]] += self_extra
    m = m.astype(E4M3)
    # slot s = b*128 + p  ->  [p, b, f]
    m = np.ascontiguousarray(m.reshape(B, cfg.PW, F).transpose(1, 0, 2))
    return m.reshape(cfg.PW, B * F)


def unpack_feature_major(cfg: Config, tab, rows, node_of):
    """rows [F, SHARD_PAD] device output -> scatter into full [N, F]
    table by orig node id (cores own disjoint node sets)."""
    a = np.asarray(rows, dtype=np.float32).T       # [SHARD_PAD, F]
    valid = node_of >= 0
    tab[node_of[valid]] = a[valid]


def build_dense(cfg: Config):
    """NEFF-0: h1 = xT.T @ W1 per shard (xT pre-scaled by dis on host)."""
    import concourse.bacc as bacc
    import concourse.mybir as mybir
    from concourse import tile

    dt = mybir.dt
    AF = mybir.ActivationFunctionType
    NPW, PW, F0, F1 = cfg.NPW, cfg.PW, cfg.F0, cfg.F1

    nc = bacc.Bacc("TRN2", target_bir_lowering=False, debug=False,
                   num_devices=cfg.NC)
    nc.move_matmul_waits_to_ldweights = lambda: None
    xT = nc.dram_tensor("xT", [F0, cfg.SHARD_PAD], dt.bfloat16,
                        kind="ExternalInput")
    W1t = nc.dram_tensor("W1t", [F0, F1], dt.bfloat16, kind="ExternalInput")
    h1 = nc.dram_tensor("h1", [F1, cfg.SHARD_PAD], dt.bfloat16,
                        kind="ExternalOutput")

    GW = 4    # windows per matmul (512 moving cols)
    with tile.TileContext(nc) as tc:
        with (
            tc.tile_pool(name="const", bufs=1) as constp,
            tc.tile_pool(name="ps", bufs=4, space="PSUM") as psp,
        ):
            w1s = constp.tile([F0, F1], dt.bfloat16)
            nc.sync.dma_start(w1s[:, :], W1t[:, :])
            xt = constp.tile([128, cfg.SHARD_PAD], dt.bfloat16)
            # small leading chunk (on scalar, parallel to W1 on sync) so
            # the first matmul starts early
            segs = [(0, 512)]
            a = 512
            while a < cfg.SHARD_PAD:
                b = min(cfg.SHARD_PAD, a + 1024)
                segs.append((a, b - a))
                a = b
            for i, (a, n) in enumerate(segs):
                eng = (nc.scalar, nc.sync)[i % 2]
                eng.dma_start(xt[:, a:a + n], xT[:, a:a + n])
            h_full = constp.tile([F1, cfg.SHARD_PAD], dt.bfloat16)
            wrote = 0
            for g0 in range(0, NPW, GW):
                gn = min(GW, NPW - g0)
                ps = psp.tile([F1, GW * PW], dt.float32, tag="ps")
                nc.tensor.matmul(out=ps[:, :gn * PW], lhsT=w1s[:, :],
                                 rhs=xt[:, g0 * PW:(g0 + gn) * PW],
                                 start=True, stop=True)
                if (g0 // GW) % 2 == 0:
                    nc.scalar.activation(
                        h_full[:, g0 * PW:(g0 + gn) * PW],
                        ps[:, :gn * PW], AF.Copy)
                else:
                    nc.vector.tensor_copy(
                        h_full[:, g0 * PW:(g0 + gn) * PW],
                        ps[:, :gn * PW])
                done = g0 + gn
                if done - wrote >= 16 or done == NPW:
                    eng = (nc.sync, nc.scalar)[(wrote // 16) % 2]
                    eng.dma_start(h1[:, wrote * PW:done * PW],
                                  h_full[:, wrote * PW:done * PW])
                    wrote = done
    _dedup_ldweights(nc)
    nc.compile()
    return nc


def build_edge(cfg: Config, sch, layer):
    """NEFF-A (layer=1): fp8 DoubleRow aggregation + relu -> z';
         XBAR pair transposes; T2 = z'^T.T @ W2'   -> h2 [F2, nodes] bf16
       NEFF-B (layer=2): fp8 DoubleRow aggregation + (1/S2) copy
                                                    -> out [128, NPW*F2]
    """
    import concourse.bacc as bacc
    import concourse.mybir as mybir
    from concourse import tile
    from concourse.masks import make_identity

    dt = mybir.dt
    AF = mybir.ActivationFunctionType
    DR = mybir.MatmulPerfMode.DoubleRow
    NPW, PW = cfg.NPW, cfg.PW
    F1, F2 = cfg.F1, cfg.F2
    FM = F1 if layer == 1 else F2
    groups, B, maxKG = sch["groups"], sch["B"], sch["maxKG"]
    TB = cfg.TB1 if layer == 1 else cfg.TB2

    nc = bacc.Bacc("TRN2", target_bir_lowering=False, debug=False,
                   num_devices=cfg.NC)
    nc.move_matmul_waits_to_ldweights = lambda: None

    msgs = nc.dram_tensor("msgs", [128, B * FM], dt.float8e4,
                          kind="ExternalInput")
    id2 = nc.dram_tensor("id2", [128, 2 * 128], dt.float8e4,
                         kind="ExternalInput")   # stacked identity
    if layer == 1:
        W2t = nc.dram_tensor("W2t", [128, F2], dt.bfloat16,
                             kind="ExternalInput")  # W2/S1 at rows 0:48,64:112
        h2 = nc.dram_tensor("h2", [F2, cfg.SHARD_PAD], dt.bfloat16,
                            kind="ExternalOutput")
    else:
        inv = nc.dram_tensor("inv", [PW, 1], dt.float32,
                             kind="ExternalInput")  # 1/S2
        out = nc.dram_tensor("out", [128, NPW * F2], dt.bfloat16,
                             kind="ExternalOutput")

    dve_groups = set()    # DVE tensor_reduce measured 3.5ns/elem: unusable
    with tile.TileContext(nc) as tc:
        with (
            tc.tile_pool(name="const", bufs=1) as constp,
            tc.tile_pool(name="zr", bufs=2) as zrp,
            tc.tile_pool(name="ps", bufs=3, space="PSUM") as psp,
            tc.tile_pool(name="psw", bufs=2, space="PSUM") as pswp,
        ):
            # The whole fp8 stream fits in SBUF: fetch EVERY group upfront
            # into exact-size tiles (no buffer reuse, so the sync queue
            # streams back-to-back with zero waits).
            gtiles = []
            for g in groups:
                n = g["KG"] * g["gn"] * FM
                gt = constp.tile([128, n], dt.float8e4)
                s0 = g["start"] * FM
                nc.sync.dma_start(gt[:, :], msgs[:, s0:s0 + n])
                gtiles.append(gt)

            ident2 = constp.tile([128, 2, 128], dt.float8e4)
            nc.scalar.dma_start(ident2[:, :, :], id2[:, :])
            if layer == 1:
                w2s = constp.tile([128, F2], dt.bfloat16)
                nc.scalar.dma_start(w2s[:, :], W2t[:, :])
                z_all = constp.tile([128, NPW * 64], dt.bfloat16)
                zT = constp.tile([128, (NPW // 2) * 128], dt.bfloat16)
                t2_all = constp.tile([F2, cfg.SHARD_PAD], dt.bfloat16)
                t2q = t2_all[:, :].rearrange("a (j rq) -> a j rq", rq=256)
            else:
                invs = constp.tile([PW, 1], dt.float32)
                nc.scalar.dma_start(invs[:, :], inv[:, :])
                o_full = constp.tile([128, NPW * F2], dt.bfloat16)

            def emit_w2(p_lo, p_hi):
                """T2 = z'^T.T @ W2' over transposed pairs [p_lo, p_hi):
                evens then odds series, 512-col chunks, PSUM->t2_all on
                vector, then flush exactly these windows to DRAM."""
                ccols = (p_hi - p_lo) * 128
                for half, r0 in ((0, 0), (1, 64)):
                    for c0 in range(0, ccols, 512):
                        cw = min(512, ccols - c0)
                        ps2 = pswp.tile([F2, 512], dt.float32, tag="ps2")
                        nc.tensor.matmul(
                            out=ps2[:, :cw],
                            lhsT=w2s[r0:r0 + F1, :],
                            rhs=zT[r0:r0 + F1,
                                   p_lo * 128 + c0:p_lo * 128 + c0 + cw],
                            start=True, stop=True)
                        npc = cw // 128
                        j0 = p_lo + c0 // 128
                        dv = t2q[:, j0:j0 + npc,
                                 half * 128:half * 128 + 128]
                        sv = ps2[:, :cw].rearrange("a (j q) -> a j q", q=128)
                        nc.vector.tensor_copy(dv, sv)
                nc.gpsimd.dma_start(h2[:, p_lo * 256:p_hi * 256],
                                    t2_all[:, p_lo * 256:p_hi * 256])

            wrote = [NPW]   # layer-2 flush high-water (window ids descend)
            pend = [None, None]   # transposed-but-not-W2'd pair range
            ready = [None]        # pairs transposed through groups <= gi-1
            for gi, g in enumerate(groups):
                gn, KG, w0 = g["gn"], g["KG"], g["w0"]
                gt = gtiles[gi]
                cols = gn * FM
                gv = gt[:, :].rearrange("p (k c) -> p k c", c=cols)
                if gi in dve_groups:
                    # DVE strided-k reduce keeps the PE free
                    acc = zrp.tile([128, TB * FM], dt.float32, tag="zr")
                    nc.vector.tensor_reduce(
                        acc[:, :cols], gv.rearrange("p k c -> p c k"),
                        mybir.AxisListType.X, mybir.AluOpType.add)
                else:
                    acc = psp.tile([128, TB * FM], dt.float32, tag="ps")
                    for k in range(0, KG, 2):
                        nc.tensor.matmul(out=acc[:, :cols],
                                         lhsT=ident2[:, :, :],
                                         rhs=gv[:, k:k + 2, :],
                                         start=(k == 0), stop=(k == KG - 2),
                                         perf_mode=DR)
                if layer == 1:
                    # relu -> z' into 64-aligned window slots (one ACT)
                    zv = z_all[:, w0 * 64:(w0 + gn) * 64].rearrange(
                        "p (w f) -> p w f", f=64)[:, :, 0:F1]
                    pv = acc[:, :cols].rearrange("p (w f) -> p w f", f=F1)
                    nc.scalar.activation(zv, pv, AF.Relu)
                    # XBAR pair transposes: [128, gn*64] -> gn/2 slabs
                    p0 = w0 // 2
                    npair = gn // 2
                    tv = zT[:, p0 * 128:(p0 + npair) * 128].rearrange(
                        "p (j q) -> p j q", q=128)
                    nc.scalar.dma_start(tv,
                                        z_all[:, w0 * 64:(w0 + gn) * 64],
                                        transpose=True)
                    # W2 lags one group so its XBAR wait never stalls the
                    # in-order PE queue
                    if pend[1] is None:
                        pend[1] = p0 + npair
                    pend[0] = p0
                    ready[0] = pend[0] + npair   # exclude current group
                    if pend[1] - ready[0] >= 8:
                        emit_w2(ready[0], pend[1])
                        pend[1] = ready[0]
                else:
                    ov = o_full[:, w0 * F2:(w0 + gn) * F2]
                    if gi in dve_groups:
                        nc.vector.tensor_scalar_mul(ov, acc[:, :cols],
                                                    invs[:, :])
                    elif gi % 2 == 0:
                        nc.vector.tensor_scalar_mul(ov, acc[:, :cols],
                                                    invs[:, :])
                    else:
                        nc.scalar.activation(ov, acc[:, :cols], AF.Copy,
                                             scale=invs[:, :])
                    hi = wrote[0]
                    if hi - w0 >= 24 or gi == len(groups) - 1:
                        nc.scalar.dma_start(out[:, w0 * F2:hi * F2],
                                            o_full[:, w0 * F2:hi * F2])
                        wrote[0] = w0
            if layer == 1 and pend[1] is not None and pend[1] > pend[0]:
                emit_w2(pend[0], pend[1])
    _dedup_ldweights(nc)
    nc.compile()
    return nc


EXEC_LOG = []  # (exec_time_ns, trace_path) per launch when BASS_TRACE=1


def run_spmd(cfg: Config, nc, in_maps):
    from concourse.bass_utils import run_bass_kernel_spmd
    res = run_bass_kernel_spmd(nc, in_maps=in_maps,
                               core_ids=list(range(cfg.NC)))
    trace_path = None
    if res.instructions_and_trace is not None:
        trace_path = res.instructions_and_trace[1]
    EXEC_LOG.append((res.exec_time_ns, trace_path))
    return res.results


def kernel(x, edge_index, W1, b1, W2, b2):
    cfg = CFG
    N, NC, PW, NPW = cfg.N, cfg.NC, cfg.PW, cfg.NPW
    meta = preprocess(cfg, edge_index)
    dis, dis2 = meta["dis"], meta["dis2"]
    sqd = 1.0 / dis

    x = np.asarray(x, dtype=np.float32)
    xs = x * dis[:, None]
    b1 = np.asarray(b1, dtype=np.float32).reshape(1, cfg.F1)
    b2 = np.asarray(b2, dtype=np.float32).reshape(1, cfg.F2)

    in0 = []
    for c in range(NC):
        nod = meta["node_of"][c]
        valid = nod >= 0
        xc = np.zeros((cfg.SHARD_PAD, cfg.F0), dtype=np.float32)
        xc[valid] = xs[nod[valid]]
        xT = np.ascontiguousarray(xc.T).astype(BF16)
        in0.append({"xT": xT, "W1t": _to_bf16(W1)})

    nc0 = build_dense(cfg)
    res0 = run_spmd(cfg, nc0, in0)
    T1 = np.zeros((N, cfg.F1), dtype=np.float32)
    for c in range(NC):
        unpack_feature_major(cfg, T1, res0[c]["h1"], meta["node_of"][c])

    # S1: max |stream value| = max(dis2_v*|T1[u]|, dis2_v*|T1[v]+sqd_v*b1|)
    sch1 = meta["sched1"]
    rmax1 = np.abs(T1).max(axis=1)
    selfv1 = T1 + sqd[:, None] * b1
    mx = 0.0
    for c in range(NC):
        sid, dsc = sch1["sid"][c], sch1["dsc"][c]
        v = sid >= 0
        m = float((np.abs(rmax1[sid[v]]) * dsc[v]).max())
        mx = max(mx, m)
    mx = max(mx, float((dis2[:, None] * np.abs(selfv1)).max()))
    S1 = pow2_gain(mx)

    ncA = build_edge(cfg, sch1, layer=1)
    eye = np.eye(128, dtype=np.float32)
    id2 = np.concatenate([eye, eye], axis=1).astype(E4M3)
    w2dup = np.zeros((128, cfg.F2), dtype=np.float32)
    w2v = np.asarray(W2, dtype=np.float32) / S1
    w2dup[0:cfg.F1] = w2v
    w2dup[64:64 + cfg.F1] = w2v
    inA = []
    for c in range(NC):
        nod = meta["node_of"][c]
        valid = nod >= 0
        ext = np.zeros((cfg.SHARD_PAD, cfg.F1), dtype=np.float32)
        ext[valid] = (S1 * dis[nod[valid], None]) * b1
        inA.append({"msgs": gather_stream(cfg, sch1, c, T1, cfg.F1, S1, ext),
                    "W2t": _to_bf16(w2dup), "id2": id2})
    resA = run_spmd(cfg, ncA, inA)
    T2 = np.zeros((N, cfg.F2), dtype=np.float32)
    for c in range(NC):
        unpack_feature_major(cfg, T2, resA[c]["h2"], meta["node_of"][c])

    sch2 = meta["sched2"]
    rmax2 = np.abs(T2).max(axis=1)
    selfv2 = dis[:, None] * T2 + b2
    mx = 0.0
    for c in range(NC):
        sid, dsc = sch2["sid"][c], sch2["dsc"][c]
        v = sid >= 0
        m = float((np.abs(rmax2[sid[v]]) * dsc[v]).max())
        mx = max(mx, m)
    mx = max(mx, float(np.abs(selfv2).max()))
    S2 = pow2_gain(mx)

    ncB = build_edge(cfg, sch2, layer=2)
    inB = []
    invv = np.full((PW, 1), 1.0 / S2, dtype=np.float32)
    for c in range(NC):
        nod = meta["node_of"][c]
        valid = nod >= 0
        # self slot extra: dsc already carries dis_v; slot = S2*dis_v*T2[v]
        # + S2*b2  (dis*sqd = 1)
        ext = np.zeros((cfg.SHARD_PAD, cfg.F2), dtype=np.float32)
        ext[valid] = S2 * b2
        inB.append({"msgs": gather_stream(cfg, sch2, c, T2, cfg.F2, S2, ext),
                    "inv": invv, "id2": id2})
    resB = run_spmd(cfg, ncB, inB)

    out = np.zeros((N, cfg.F2), dtype=np.float32)
    for c in range(NC):
        rows = np.asarray(resB[c]["out"]).astype(np.float32)
        a = rows.reshape(cfg.PW, NPW, cfg.F2).transpose(1, 0, 2).reshape(
            -1, cfg.F2)
        nod = meta["node_of"][c]
        valid = nod >= 0
        out[nod[valid]] = a[valid]
    return out


# revision 15
# speedup vs baseline: 1.7721x; 1.5083x over previous
"""Two-layer GCN (PyG gcn_norm semantics) on 8 Trainium2 NeuronCores.

v2: fp8 DoubleRow identity-scatter (graph/data parallel, dst-sharded,
host-transported):

  - norm factorizes: norm(u->v) = dis[u]*dis[v], dis = (deg_in+1)^-1/2.
    Host pre-scales every edge message by its DESTINATION factor so the
    device epilogues are plain relu/copy (no per-window scale ops):
      L1 slot value = S1*dis2_v*T1[u]           (self: +S1*dis_v*b1)
      L2 slot value = S2*dis_v*T2[u]            (self: +S2*b2)
    with T1 = dis*(x@W1), T2 = z'@(W2/S1), z' = S1*dis*z. S1/S2 are
    power-of-two gains keeping fp8 e4m3 values in the normal range;
    1/S1 folds into the W2 weights, 1/S2 into the final output copy.

  - Streams are fp8 e4m3, aggregated with DoubleRow matmuls against a
    stacked identity: one matmul PSUM-accumulates TWO 128-edge blocks at
    0.5 cycles/row (4x bf16 throughput). Group k-depth is forced even.

  - Layer-1 tail: relu writes z' into 64-aligned window slots; XBAR DMA
    transposes window pairs SBUF->SBUF (no PE); W2 matmuls run with W2
    stationary and z'^T moving (512 node-cols per matmul), producing the
    T2 table feature-major for free host untransposition.

  - Three launches:
      NEFF-0: h1 = (dis*x) @ W1 per shard          -> [F1, nodes] bf16
      host:   gather+scale T1[src] into fp8 slot streams
      NEFF-A: L1 aggregation + relu + T2 = z'@W2'  -> [F2, nodes] bf16
      host:   gather+scale T2[src] (fp8)
      NEFF-B: L2 aggregation + 1/S2 copy           -> out bf16
"""

from dataclasses import dataclass

import numpy as np
import ml_dtypes

BF16 = ml_dtypes.bfloat16
E4M3 = ml_dtypes.float8_e4m3


@dataclass
class Config:
    N: int = 100000          # nodes
    F0: int = 128            # input features
    F1: int = 48             # hidden
    F2: int = 32             # out
    NC: int = 8              # cores
    PW: int = 128            # window (nodes per PSUM window)
    TB1: int = 10            # windows per group, layer 1 (TB1*F1 <= 512)
    TB2: int = 16            # windows per group, layer 2 (TB2*F2 <= 512)

    @property
    def NW(self):            # global windows (multiple of NC)
        nw = (self.N + self.PW - 1) // self.PW
        return ((nw + self.NC - 1) // self.NC) * self.NC

    @property
    def NPW(self):           # windows per core
        return self.NW // self.NC

    @property
    def SHARD_PAD(self):
        return self.NPW * self.PW


CFG = Config()


def _to_bf16(a):
    return np.asarray(a, dtype=np.float32).astype(BF16)


def _dedup_ldweights(nc):
    """Delete redundant InstLdweights: the PE array keeps its stationary
    matrix across matmuls, so a reload of the identical weights (and no
    semaphore wait/update riding on it) is dead work."""
    import concourse.mybir as mybir
    ndel = 0
    for fn in nc.m.functions:
        for blk in fn.blocks:
            keep, last_sig = [], None
            for inst in blk.instructions:
                if isinstance(inst, mybir.InstLdweights):
                    sig = inst.concise(deps=False)
                    if (sig == last_sig and not inst.has_wait()
                            and not inst.has_update()):
                        ndel += 1
                        continue
                    last_sig = sig
                elif (not isinstance(inst, mybir.InstMatmult)
                      and getattr(inst, "engine", None) == mybir.EngineType.PE
                      and inst.is_executable()):
                    last_sig = None
                keep.append(inst)
            blk.instructions = keep
    return ndel


def make_sched(cfg: Config, nb, TB, F, even_gn):
    """Partition consecutive windows into groups (DP-optimized sizes up
    to TB) with per-group even k-depth KG = even(max nb). Stream layout
    is k-major per group ([k][w][F]). Groups are listed (and processed)
    in descending-id = ascending-degree order."""
    NPW = cfg.NPW
    OH, CYC = 350.0, 0.83     # measured per-matmul overhead / PE ns-cycle
    step = 2 if even_gn else 1
    INF = float("inf")
    best = [INF] * (NPW + 1)
    best[0] = 0.0
    choice = [0] * (NPW + 1)
    for i in range(1, NPW + 1):
        mx = 0
        for gn in range(1, min(TB, i) + 1):
            mx = max(mx, int(nb[i - gn]))
            if even_gn and gn % 2:
                continue
            KG = mx + (mx % 2)
            cost = (KG // 2) * (OH + gn * F * 0.5 * CYC)
            if best[i - gn] + cost < best[i]:
                best[i] = best[i - gn] + cost
                choice[i] = gn
    bounds = []
    i = NPW
    while i > 0:
        gn = choice[i]
        bounds.append((i - gn, gn))
        i -= gn
    # bounds already descending by window id (ascending degree)
    groups = []
    maxnb = int(max(nb))
    lut = np.full((NPW, maxnb), -1, dtype=np.int64)
    blk = 0
    for lo, gn in bounds:
        wins = list(range(lo, lo + gn))     # ascending ids
        KG = max(int(nb[w]) for w in wins)
        KG += KG % 2                        # force even for DoubleRow
        for wi, w in enumerate(wins):
            for k in range(int(nb[w])):
                lut[w, k] = blk + k * gn + wi
        groups.append({"wins": wins, "w0": lo, "gn": gn, "KG": KG,
                       "start": blk})
        blk += gn * KG
    return {"groups": groups, "lut": lut, "B": blk, "maxKG": max(
        g["KG"] for g in groups)}


def preprocess(cfg: Config, edge_index):
    N, NC, PW, NPW = cfg.N, cfg.NC, cfg.PW, cfg.NPW
    NW = cfg.NW

    src = np.asarray(edge_index[0], dtype=np.int64)
    dst = np.asarray(edge_index[1], dtype=np.int64)
    E = src.shape[0]

    indeg = np.bincount(dst, minlength=N)
    degp1 = indeg.astype(np.float64) + 1.0
    dis = (degp1 ** -0.5).astype(np.float32)
    dis2 = (degp1 ** -1.0).astype(np.float32)

    perm = np.argsort(-indeg, kind="stable")       # rank -> orig node
    rank = np.empty(N, dtype=np.int64)
    rank[perm] = np.arange(N)

    indeg_sorted = indeg[perm]                     # descending
    win_max = np.zeros(NW, dtype=np.int64)
    nwin_real = (N + PW - 1) // PW
    win_max[:nwin_real] = indeg_sorted[::PW][:nwin_real]
    nb = 1 + win_max.reshape(NPW, NC).max(axis=1)  # shared schedule [NPW]

    # node at (core c, local window g, pos p) = perm[(g*NC + c)*PW + p]
    node_of = []
    slots_all = np.full(NW * PW, -1, dtype=np.int64)
    slots_all[:N] = perm
    grid = slots_all.reshape(NPW, NC, PW)          # [g, c, p]
    for c in range(NC):
        node_of.append(np.ascontiguousarray(grid[:, c, :]).reshape(-1))

    # per-edge position: k-th in-edge (k starting at 1; 0 = self)
    rd = rank[dst]
    order_e = np.argsort(rd, kind="stable")
    src_s = src[order_e]
    rd_s = rd[order_e]
    cum = np.concatenate([[0], np.cumsum(indeg_sorted)])
    k_e = np.arange(E) - cum[rd_s] + 1             # 1..indeg
    wg = rd_s // PW
    p_e = rd_s % PW
    g_e = wg // NC                                 # local window
    c_e = wg % NC                                  # core
    dis_r = dis[perm]                              # by rank
    dis2_r = dis2[perm]

    meta = {"nb": nb, "node_of": node_of, "dis": dis, "dis2": dis2,
            "perm": perm}

    for layer, TB in ((1, cfg.TB1), (2, cfg.TB2)):
        sch = make_sched(cfg, nb, TB, cfg.F1 if layer == 1 else cfg.F2,
                         even_gn=(layer == 1))
        lut, B = sch["lut"], sch["B"]
        slot_e = lut[g_e, k_e] * PW + p_e
        dsc_e = (dis2_r if layer == 1 else dis_r)[rd_s]
        self_blocks = lut[:, 0]                    # [NPW]
        self_slots = (self_blocks[:, None] * PW
                      + np.arange(PW)[None, :]).reshape(-1)
        sid_c, dsc_c = [], []
        for c in range(NC):
            sid = np.full(B * PW, -1, dtype=np.int64)
            dsc = np.zeros(B * PW, dtype=np.float32)
            m = c_e == c
            sid[slot_e[m]] = src_s[m]
            dsc[slot_e[m]] = dsc_e[m]
            nod = node_of[c]
            valid = nod >= 0
            sv = np.zeros(cfg.SHARD_PAD, dtype=np.float32)
            sv[valid] = (dis2 if layer == 1 else dis)[nod[valid]]
            sid[self_slots] = nod
            dsc[self_slots] = sv
            sid_c.append(sid)
            dsc_c.append(dsc)
        sch["sid"] = sid_c
        sch["dsc"] = dsc_c
        sch["self_slots"] = self_slots
        meta[f"sched{layer}"] = sch
    return meta


def pow2_gain(mx, target=240.0):
    if mx <= 0:
        return 1.0
    return float(2.0 ** np.floor(np.log2(target / mx)))


def gather_stream(cfg: Config, sch, c, table, F, S, self_extra):
    """table [N, F] f32 -> [128, B*F] e4m3 slot stream for core c.
    Slot value = S * dsc[slot] * table[sid[slot]]; self_extra [SHARD_PAD, F]
    (S * per-node bias term) is added onto the self-loop slots."""
    sid, dsc = sch["sid"][c], sch["dsc"][c]
    B = sid.shape[0] // cfg.PW
    m = np.zeros((sid.shape[0], F), dtype=np.float32)
    valid = sid >= 0
    m[valid] = table[sid[valid]] * (dsc[valid] * S)[:, None]
    if self_extra is not None:
        m[sch["self_slots"]] += self_extra
    m = m.astype(E4M3)
    # slot s = b*128 + p  ->  [p, b, f]
    m = np.ascontiguousarray(m.reshape(B, cfg.PW, F).transpose(1, 0, 2))
    return m.reshape(cfg.PW, B * F)


def unpack_feature_major(cfg: Config, tab, rows, node_of):
    """rows [F, SHARD_PAD] device output -> scatter into full [N, F]
    table by orig node id (cores own disjoint node sets)."""
    a = np.asarray(rows, dtype=np.float32).T       # [SHARD_PAD, F]
    valid = node_of >= 0
    tab[node_of[valid]] = a[valid]


def build_dense(cfg: Config):
    """NEFF-0: h1 = xT.T @ W1 per shard (xT pre-scaled by dis on host)."""
    import concourse.bacc as bacc
    import concourse.mybir as mybir
    from concourse import tile

    dt = mybir.dt
    AF = mybir.ActivationFunctionType
    NPW, PW, F0, F1 = cfg.NPW, cfg.PW, cfg.F0, cfg.F1

    nc = bacc.Bacc("TRN2", target_bir_lowering=False, debug=False,
                   num_devices=cfg.NC)
    nc.move_matmul_waits_to_ldweights = lambda: None
    xT = nc.dram_tensor("xT", [F0, cfg.SHARD_PAD], dt.bfloat16,
                        kind="ExternalInput")
    W1t = nc.dram_tensor("W1t", [F0, F1], dt.bfloat16, kind="ExternalInput")
    h1 = nc.dram_tensor("h1", [F1, cfg.SHARD_PAD], dt.bfloat16,
                        kind="ExternalOutput")

    GW = 4    # windows per matmul (512 moving cols)
    with tile.TileContext(nc) as tc:
        with (
            tc.tile_pool(name="const", bufs=1) as constp,
            tc.tile_pool(name="ps", bufs=4, space="PSUM") as psp,
        ):
            w1s = constp.tile([F0, F1], dt.bfloat16)
            nc.scalar.dma_start(w1s[:, :], W1t[:, :])
            xt = constp.tile([128, cfg.SHARD_PAD], dt.bfloat16)
            # 3 big chunks: small leader (sync), middle (gpsimd SWDGE),
            # tail (sync) — consecutive DMAs on one queue have a ~5.5us
            # gap, so few big chunks on two queues deliver fastest
            mid = (cfg.SHARD_PAD // 2 // PW) * PW
            nc.sync.dma_start(xt[:, 0:1024], xT[:, 0:1024])
            nc.gpsimd.dma_start(xt[:, 1024:mid], xT[:, 1024:mid])
            nc.sync.dma_start(xt[:, mid:], xT[:, mid:])
            h_full = constp.tile([F1, cfg.SHARD_PAD], dt.bfloat16)
            wrote = 0
            for g0 in range(0, NPW, GW):
                gn = min(GW, NPW - g0)
                ps = psp.tile([F1, GW * PW], dt.float32, tag="ps")
                nc.tensor.matmul(out=ps[:, :gn * PW], lhsT=w1s[:, :],
                                 rhs=xt[:, g0 * PW:(g0 + gn) * PW],
                                 start=True, stop=True)
                if (g0 // GW) % 2 == 0:
                    nc.scalar.activation(
                        h_full[:, g0 * PW:(g0 + gn) * PW],
                        ps[:, :gn * PW], AF.Copy)
                else:
                    nc.vector.tensor_copy(
                        h_full[:, g0 * PW:(g0 + gn) * PW],
                        ps[:, :gn * PW])
                done = g0 + gn
                if done - wrote >= 48 or done == NPW:
                    nc.sync.dma_start(h1[:, wrote * PW:done * PW],
                                      h_full[:, wrote * PW:done * PW])
                    wrote = done
    _dedup_ldweights(nc)
    nc.compile()
    return nc


def build_edge(cfg: Config, sch, layer):
    """NEFF-A (layer=1): fp8 DoubleRow aggregation + relu -> z';
         XBAR pair transposes; T2 = z'^T.T @ W2'   -> h2 [F2, nodes] bf16
       NEFF-B (layer=2): fp8 DoubleRow aggregation + (1/S2) copy
                                                    -> out [128, NPW*F2]
    """
    import concourse.bacc as bacc
    import concourse.mybir as mybir
    from concourse import tile
    from concourse.masks import make_identity

    dt = mybir.dt
    AF = mybir.ActivationFunctionType
    DR = mybir.MatmulPerfMode.DoubleRow
    NPW, PW = cfg.NPW, cfg.PW
    F1, F2 = cfg.F1, cfg.F2
    FM = F1 if layer == 1 else F2
    groups, B, maxKG = sch["groups"], sch["B"], sch["maxKG"]
    TB = cfg.TB1 if layer == 1 else cfg.TB2

    nc = bacc.Bacc("TRN2", target_bir_lowering=False, debug=False,
                   num_devices=cfg.NC)
    nc.move_matmul_waits_to_ldweights = lambda: None

    msgs = nc.dram_tensor("msgs", [128, B * FM], dt.float8e4,
                          kind="ExternalInput")
    id2 = nc.dram_tensor("id2", [128, 2 * 128], dt.float8e4,
                         kind="ExternalInput")   # stacked identity
    if layer == 1:
        W2t = nc.dram_tensor("W2t", [128, F2], dt.bfloat16,
                             kind="ExternalInput")  # W2/S1 at rows 0:48,64:112
        h2 = nc.dram_tensor("h2", [F2, cfg.SHARD_PAD], dt.bfloat16,
                            kind="ExternalOutput")
    else:
        inv = nc.dram_tensor("inv", [PW, 1], dt.float32,
                             kind="ExternalInput")  # 1/S2
        out = nc.dram_tensor("out", [128, NPW * F2], dt.bfloat16,
                             kind="ExternalOutput")

    dve_groups = set()    # DVE tensor_reduce measured 3.5ns/elem: unusable
    with tile.TileContext(nc) as tc:
        with (
            tc.tile_pool(name="const", bufs=1) as constp,
            tc.tile_pool(name="zr", bufs=2) as zrp,
            tc.tile_pool(name="ps", bufs=3, space="PSUM") as psp,
            tc.tile_pool(name="psw", bufs=2, space="PSUM") as pswp,
        ):
            # The whole fp8 stream fits in SBUF. Fetch it upfront into ONE
            # resident tile via a few BIG chunk DMAs: consecutive DMAs on
            # one queue have a ~5.5us dead gap between transfers, so use
            # few chunks (>=2MB after the first) alternating between the
            # sync (HWDGE) and gpsimd (SWDGE) queues; scalar stays clean
            # for the relu/XBAR chain.
            stream = constp.tile([128, B * FM], dt.float8e4)
            bounds = [g["start"] * FM for g in groups] + [B * FM]
            chunks = []
            lo = 0
            for gi in range(1, len(groups) + 1):
                target = 4000 if not chunks else 18000
                if bounds[gi] - lo >= target or gi == len(groups):
                    chunks.append((lo, bounds[gi]))
                    lo = bounds[gi]
            for ci, (a, b) in enumerate(chunks):
                eng = (nc.sync, nc.gpsimd)[ci % 2]
                eng.dma_start(stream[:, a:b], msgs[:, a:b])

            ident2 = constp.tile([128, 2, 128], dt.float8e4)
            nc.scalar.dma_start(ident2[:, :, :], id2[:, :])
            if layer == 1:
                w2s = constp.tile([128, F2], dt.bfloat16)
                nc.scalar.dma_start(w2s[:, :], W2t[:, :])
                z_all = constp.tile([128, NPW * 64], dt.bfloat16)
                zT = constp.tile([128, (NPW // 2) * 128], dt.bfloat16)
                t2_all = constp.tile([F2, cfg.SHARD_PAD], dt.bfloat16)
                t2q = t2_all[:, :].rearrange("a (j rq) -> a j rq", rq=256)
            else:
                invs = constp.tile([PW, 1], dt.float32)
                nc.scalar.dma_start(invs[:, :], inv[:, :])
                o_full = constp.tile([128, NPW * F2], dt.bfloat16)

            def emit_w2(p_lo, p_hi):
                """T2 = z'^T.T @ W2' over transposed pairs [p_lo, p_hi):
                evens then odds series, 512-col chunks, PSUM->t2_all on
                vector, then flush exactly these windows to DRAM."""
                ccols = (p_hi - p_lo) * 128
                for half, r0 in ((0, 0), (1, 64)):
                    for c0 in range(0, ccols, 512):
                        cw = min(512, ccols - c0)
                        ps2 = pswp.tile([F2, 512], dt.float32, tag="ps2")
                        nc.tensor.matmul(
                            out=ps2[:, :cw],
                            lhsT=w2s[r0:r0 + F1, :],
                            rhs=zT[r0:r0 + F1,
                                   p_lo * 128 + c0:p_lo * 128 + c0 + cw],
                            start=True, stop=True)
                        npc = cw // 128
                        j0 = p_lo + c0 // 128
                        dv = t2q[:, j0:j0 + npc,
                                 half * 128:half * 128 + 128]
                        sv = ps2[:, :cw].rearrange("a (j q) -> a j q", q=128)
                        nc.vector.tensor_copy(dv, sv)
                nc.gpsimd.dma_start(h2[:, p_lo * 256:p_hi * 256],
                                    t2_all[:, p_lo * 256:p_hi * 256])

            wrote = [NPW]   # layer-2 flush high-water (window ids descend)
            pend = [None, None]   # transposed-but-not-W2'd pair range
            ready = [None]        # pairs transposed through groups <= gi-1
            for gi, g in enumerate(groups):
                gn, KG, w0 = g["gn"], g["KG"], g["w0"]
                cols = gn * FM
                s0 = g["start"] * FM
                gv = stream[:, s0:s0 + KG * cols].rearrange(
                    "p (k c) -> p k c", c=cols)
                if gi in dve_groups:
                    # DVE strided-k reduce keeps the PE free
                    acc = zrp.tile([128, TB * FM], dt.float32, tag="zr")
                    nc.vector.tensor_reduce(
                        acc[:, :cols], gv.rearrange("p k c -> p c k"),
                        mybir.AxisListType.X, mybir.AluOpType.add)
                else:
                    acc = psp.tile([128, TB * FM], dt.float32, tag="ps")
                    for k in range(0, KG, 2):
                        nc.tensor.matmul(out=acc[:, :cols],
                                         lhsT=ident2[:, :, :],
                                         rhs=gv[:, k:k + 2, :],
                                         start=(k == 0), stop=(k == KG - 2),
                                         perf_mode=DR)
                if layer == 1:
                    # relu -> z' into 64-aligned window slots (one ACT)
                    zv = z_all[:, w0 * 64:(w0 + gn) * 64].rearrange(
                        "p (w f) -> p w f", f=64)[:, :, 0:F1]
                    pv = acc[:, :cols].rearrange("p (w f) -> p w f", f=F1)
                    nc.scalar.activation(zv, pv, AF.Relu)
                    # XBAR pair transposes: [128, gn*64] -> gn/2 slabs
                    p0 = w0 // 2
                    npair = gn // 2
                    tv = zT[:, p0 * 128:(p0 + npair) * 128].rearrange(
                        "p (j q) -> p j q", q=128)
                    nc.scalar.dma_start(tv,
                                        z_all[:, w0 * 64:(w0 + gn) * 64],
                                        transpose=True)
                    # W2 lags one group so its XBAR wait never stalls the
                    # in-order PE queue
                    if pend[1] is None:
                        pend[1] = p0 + npair
                    pend[0] = p0
                    ready[0] = pend[0] + npair   # exclude current group
                    if pend[1] - ready[0] >= 8:
                        emit_w2(ready[0], pend[1])
                        pend[1] = ready[0]
                else:
                    ov = o_full[:, w0 * F2:(w0 + gn) * F2]
                    if gi in dve_groups:
                        nc.vector.tensor_scalar_mul(ov, acc[:, :cols],
                                                    invs[:, :])
                    elif gi % 2 == 0:
                        nc.vector.tensor_scalar_mul(ov, acc[:, :cols],
                                                    invs[:, :])
                    else:
                        nc.scalar.activation(ov, acc[:, :cols], AF.Copy,
                                             scale=invs[:, :])
                    hi = wrote[0]
                    if hi - w0 >= 24 or gi == len(groups) - 1:
                        nc.scalar.dma_start(out[:, w0 * F2:hi * F2],
                                            o_full[:, w0 * F2:hi * F2])
                        wrote[0] = w0
            if layer == 1 and pend[1] is not None and pend[1] > pend[0]:
                emit_w2(pend[0], pend[1])
    _dedup_ldweights(nc)
    nc.compile()
    return nc


EXEC_LOG = []  # (exec_time_ns, trace_path) per launch when BASS_TRACE=1


def run_spmd(cfg: Config, nc, in_maps):
    from concourse.bass_utils import run_bass_kernel_spmd
    res = run_bass_kernel_spmd(nc, in_maps=in_maps,
                               core_ids=list(range(cfg.NC)))
    trace_path = None
    if res.instructions_and_trace is not None:
        trace_path = res.instructions_and_trace[1]
    EXEC_LOG.append((res.exec_time_ns, trace_path))
    return res.results


def kernel(x, edge_index, W1, b1, W2, b2):
    cfg = CFG
    N, NC, PW, NPW = cfg.N, cfg.NC, cfg.PW, cfg.NPW
    meta = preprocess(cfg, edge_index)
    dis, dis2 = meta["dis"], meta["dis2"]
    sqd = 1.0 / dis

    x = np.asarray(x, dtype=np.float32)
    xs = x * dis[:, None]
    b1 = np.asarray(b1, dtype=np.float32).reshape(1, cfg.F1)
    b2 = np.asarray(b2, dtype=np.float32).reshape(1, cfg.F2)

    in0 = []
    for c in range(NC):
        nod = meta["node_of"][c]
        valid = nod >= 0
        xc = np.zeros((cfg.SHARD_PAD, cfg.F0), dtype=np.float32)
        xc[valid] = xs[nod[valid]]
        xT = np.ascontiguousarray(xc.T).astype(BF16)
        in0.append({"xT": xT, "W1t": _to_bf16(W1)})

    nc0 = build_dense(cfg)
    res0 = run_spmd(cfg, nc0, in0)
    T1 = np.zeros((N, cfg.F1), dtype=np.float32)
    for c in range(NC):
        unpack_feature_major(cfg, T1, res0[c]["h1"], meta["node_of"][c])

    # S1: max |stream value| = max(dis2_v*|T1[u]|, dis2_v*|T1[v]+sqd_v*b1|)
    sch1 = meta["sched1"]
    rmax1 = np.abs(T1).max(axis=1)
    selfv1 = T1 + sqd[:, None] * b1
    mx = 0.0
    for c in range(NC):
        sid, dsc = sch1["sid"][c], sch1["dsc"][c]
        v = sid >= 0
        m = float((np.abs(rmax1[sid[v]]) * dsc[v]).max())
        mx = max(mx, m)
    mx = max(mx, float((dis2[:, None] * np.abs(selfv1)).max()))
    S1 = pow2_gain(mx)

    ncA = build_edge(cfg, sch1, layer=1)
    eye = np.eye(128, dtype=np.float32)
    id2 = np.concatenate([eye, eye], axis=1).astype(E4M3)
    w2dup = np.zeros((128, cfg.F2), dtype=np.float32)
    w2v = np.asarray(W2, dtype=np.float32) / S1
    w2dup[0:cfg.F1] = w2v
    w2dup[64:64 + cfg.F1] = w2v
    inA = []
    for c in range(NC):
        nod = meta["node_of"][c]
        valid = nod >= 0
        ext = np.zeros((cfg.SHARD_PAD, cfg.F1), dtype=np.float32)
        ext[valid] = (S1 * dis[nod[valid], None]) * b1
        inA.append({"msgs": gather_stream(cfg, sch1, c, T1, cfg.F1, S1, ext),
                    "W2t": _to_bf16(w2dup), "id2": id2})
    resA = run_spmd(cfg, ncA, inA)
    T2 = np.zeros((N, cfg.F2), dtype=np.float32)
    for c in range(NC):
        unpack_feature_major(cfg, T2, resA[c]["h2"], meta["node_of"][c])

    sch2 = meta["sched2"]
    rmax2 = np.abs(T2).max(axis=1)
    selfv2 = dis[:, None] * T2 + b2
    mx = 0.0
    for c in range(NC):
        sid, dsc = sch2["sid"][c], sch2["dsc"][c]
        v = sid >= 0
        m = float((np.abs(rmax2[sid[v]]) * dsc[v]).max())
        mx = max(mx, m)
    mx = max(mx, float(np.abs(selfv2).max()))
    S2 = pow2_gain(mx)

    ncB = build_edge(cfg, sch2, layer=2)
    inB = []
    invv = np.full((PW, 1), 1.0 / S2, dtype=np.float32)
    for c in range(NC):
        nod = meta["node_of"][c]
        valid = nod >= 0
        # self slot extra: dsc already carries dis_v; slot = S2*dis_v*T2[v]
        # + S2*b2  (dis*sqd = 1)
        ext = np.zeros((cfg.SHARD_PAD, cfg.F2), dtype=np.float32)
        ext[valid] = S2 * b2
        inB.append({"msgs": gather_stream(cfg, sch2, c, T2, cfg.F2, S2, ext),
                    "inv": invv, "id2": id2})
    resB = run_spmd(cfg, ncB, inB)

    out = np.zeros((N, cfg.F2), dtype=np.float32)
    for c in range(NC):
        rows = np.asarray(resB[c]["out"]).astype(np.float32)
        a = rows.reshape(cfg.PW, NPW, cfg.F2).transpose(1, 0, 2).reshape(
            -1, cfg.F2)
        nod = meta["node_of"][c]
        valid = nod >= 0
        out[nod[valid]] = a[valid]
    return out


# revision 22
# speedup vs baseline: 1.7910x; 1.0107x over previous
"""Two-layer GCN (PyG gcn_norm semantics) on 8 Trainium2 NeuronCores.

v2: fp8 DoubleRow identity-scatter (graph/data parallel, dst-sharded,
host-transported):

  - norm factorizes: norm(u->v) = dis[u]*dis[v], dis = (deg_in+1)^-1/2.
    Host pre-scales every edge message by its DESTINATION factor so the
    device epilogues are plain relu/copy (no per-window scale ops):
      L1 slot value = S1*dis2_v*T1[u]           (self: +S1*dis_v*b1)
      L2 slot value = S2*dis_v*T2[u]            (self: +S2*b2)
    with T1 = dis*(x@W1), T2 = z'@(W2/S1), z' = S1*dis*z. S1/S2 are
    power-of-two gains keeping fp8 e4m3 values in the normal range;
    1/S1 folds into the W2 weights, 1/S2 into the final output copy.

  - Streams are fp8 e4m3, aggregated with DoubleRow matmuls against a
    stacked identity: one matmul PSUM-accumulates TWO 128-edge blocks at
    0.5 cycles/row (4x bf16 throughput). Group k-depth is forced even.

  - Layer-1 tail: relu writes z' into 64-aligned window slots; XBAR DMA
    transposes window pairs SBUF->SBUF (no PE); W2 matmuls run with W2
    stationary and z'^T moving (512 node-cols per matmul), producing the
    T2 table feature-major for free host untransposition.

  - Three launches:
      NEFF-0: h1 = (dis*x) @ W1 per shard          -> [F1, nodes] bf16
      host:   gather+scale T1[src] into fp8 slot streams
      NEFF-A: L1 aggregation + relu + T2 = z'@W2'  -> [F2, nodes] bf16
      host:   gather+scale T2[src] (fp8)
      NEFF-B: L2 aggregation + 1/S2 copy           -> out bf16
"""

from dataclasses import dataclass

import numpy as np
import ml_dtypes

BF16 = ml_dtypes.bfloat16
E4M3 = ml_dtypes.float8_e4m3


@dataclass
class Config:
    N: int = 100000          # nodes
    F0: int = 128            # input features
    F1: int = 48             # hidden
    F2: int = 32             # out
    NC: int = 8              # cores
    PW: int = 128            # window (nodes per PSUM window)
    TB1: int = 10            # windows per group, layer 1 (TB1*F1 <= 512)
    TB2: int = 16            # windows per group, layer 2 (TB2*F2 <= 512)

    @property
    def NW(self):            # global windows (multiple of NC)
        nw = (self.N + self.PW - 1) // self.PW
        return ((nw + self.NC - 1) // self.NC) * self.NC

    @property
    def NPW(self):           # windows per core
        return self.NW // self.NC

    @property
    def SHARD_PAD(self):
        return self.NPW * self.PW


CFG = Config()


def _to_bf16(a):
    return np.asarray(a, dtype=np.float32).astype(BF16)


def _dedup_ldweights(nc):
    """Delete redundant InstLdweights: the PE array keeps its stationary
    matrix across matmuls, so a reload of the identical weights (and no
    semaphore wait/update riding on it) is dead work."""
    import concourse.mybir as mybir
    ndel = 0
    for fn in nc.m.functions:
        for blk in fn.blocks:
            keep, last_sig = [], None
            for inst in blk.instructions:
                if isinstance(inst, mybir.InstLdweights):
                    sig = inst.concise(deps=False)
                    if (sig == last_sig and not inst.has_wait()
                            and not inst.has_update()):
                        ndel += 1
                        continue
                    last_sig = sig
                elif (not isinstance(inst, mybir.InstMatmult)
                      and getattr(inst, "engine", None) == mybir.EngineType.PE
                      and inst.is_executable()):
                    last_sig = None
                keep.append(inst)
            blk.instructions = keep
    return ndel


def make_sched(cfg: Config, nb, TB, F, even_gn):
    """Partition consecutive windows into groups (DP-optimized sizes up
    to TB) with per-group even k-depth KG = even(max nb). Stream layout
    is k-major per group ([k][w][F]). Groups are listed (and processed)
    in descending-id = ascending-degree order."""
    NPW = cfg.NPW
    OH, CYC = 350.0, 0.83     # measured per-matmul overhead / PE ns-cycle
    step = 2 if even_gn else 1
    INF = float("inf")
    best = [INF] * (NPW + 1)
    best[0] = 0.0
    choice = [0] * (NPW + 1)
    for i in range(1, NPW + 1):
        mx = 0
        for gn in range(1, min(TB, i) + 1):
            mx = max(mx, int(nb[i - gn]))
            if even_gn and gn % 2:
                continue
            KG = mx + (mx % 2)
            cost = (KG // 2) * (OH + gn * F * 0.5 * CYC)
            if best[i - gn] + cost < best[i]:
                best[i] = best[i - gn] + cost
                choice[i] = gn
    bounds = []
    i = NPW
    while i > 0:
        gn = choice[i]
        bounds.append((i - gn, gn))
        i -= gn
    # bounds already descending by window id (ascending degree)
    groups = []
    maxnb = int(max(nb))
    lut = np.full((NPW, maxnb), -1, dtype=np.int64)
    blk = 0
    for lo, gn in bounds:
        wins = list(range(lo, lo + gn))     # ascending ids
        KG = max(int(nb[w]) for w in wins)
        KG += KG % 2                        # force even for DoubleRow
        for wi, w in enumerate(wins):
            for k in range(int(nb[w])):
                lut[w, k] = blk + k * gn + wi
        groups.append({"wins": wins, "w0": lo, "gn": gn, "KG": KG,
                       "start": blk})
        blk += gn * KG
    return {"groups": groups, "lut": lut, "B": blk, "maxKG": max(
        g["KG"] for g in groups)}


def preprocess(cfg: Config, edge_index):
    N, NC, PW, NPW = cfg.N, cfg.NC, cfg.PW, cfg.NPW
    NW = cfg.NW

    src = np.asarray(edge_index[0], dtype=np.int64)
    dst = np.asarray(edge_index[1], dtype=np.int64)
    E = src.shape[0]

    indeg = np.bincount(dst, minlength=N)
    degp1 = indeg.astype(np.float64) + 1.0
    dis = (degp1 ** -0.5).astype(np.float32)
    dis2 = (degp1 ** -1.0).astype(np.float32)

    perm = np.argsort(-indeg, kind="stable")       # rank -> orig node
    rank = np.empty(N, dtype=np.int64)
    rank[perm] = np.arange(N)

    indeg_sorted = indeg[perm]                     # descending
    win_max = np.zeros(NW, dtype=np.int64)
    nwin_real = (N + PW - 1) // PW
    win_max[:nwin_real] = indeg_sorted[::PW][:nwin_real]
    nb = 1 + win_max.reshape(NPW, NC).max(axis=1)  # shared schedule [NPW]

    # node at (core c, local window g, pos p) = perm[(g*NC + c)*PW + p]
    node_of = []
    slots_all = np.full(NW * PW, -1, dtype=np.int64)
    slots_all[:N] = perm
    grid = slots_all.reshape(NPW, NC, PW)          # [g, c, p]
    for c in range(NC):
        node_of.append(np.ascontiguousarray(grid[:, c, :]).reshape(-1))

    # per-edge position: k-th in-edge (k starting at 1; 0 = self)
    rd = rank[dst]
    order_e = np.argsort(rd, kind="stable")
    src_s = src[order_e]
    rd_s = rd[order_e]
    cum = np.concatenate([[0], np.cumsum(indeg_sorted)])
    k_e = np.arange(E) - cum[rd_s] + 1             # 1..indeg
    wg = rd_s // PW
    p_e = rd_s % PW
    g_e = wg // NC                                 # local window
    c_e = wg % NC                                  # core
    dis_r = dis[perm]                              # by rank
    dis2_r = dis2[perm]

    meta = {"nb": nb, "node_of": node_of, "dis": dis, "dis2": dis2,
            "perm": perm}

    for layer, TB in ((1, cfg.TB1), (2, cfg.TB2)):
        sch = make_sched(cfg, nb, TB, cfg.F1 if layer == 1 else cfg.F2,
                         even_gn=(layer == 1))
        lut, B = sch["lut"], sch["B"]
        slot_e = lut[g_e, k_e] * PW + p_e
        dsc_e = (dis2_r if layer == 1 else dis_r)[rd_s]
        self_blocks = lut[:, 0]                    # [NPW]
        self_slots = (self_blocks[:, None] * PW
                      + np.arange(PW)[None, :]).reshape(-1)
        sid_c, dsc_c = [], []
        for c in range(NC):
            sid = np.full(B * PW, -1, dtype=np.int64)
            dsc = np.zeros(B * PW, dtype=np.float32)
            m = c_e == c
            sid[slot_e[m]] = src_s[m]
            dsc[slot_e[m]] = dsc_e[m]
            nod = node_of[c]
            valid = nod >= 0
            sv = np.zeros(cfg.SHARD_PAD, dtype=np.float32)
            sv[valid] = (dis2 if layer == 1 else dis)[nod[valid]]
            sid[self_slots] = nod
            dsc[self_slots] = sv
            sid_c.append(sid)
            dsc_c.append(dsc)
        sch["sid"] = sid_c
        sch["dsc"] = dsc_c
        sch["self_slots"] = self_slots
        meta[f"sched{layer}"] = sch
    return meta


def pow2_gain(mx, target=240.0):
    if mx <= 0:
        return 1.0
    return float(2.0 ** np.floor(np.log2(target / mx)))


def gather_stream(cfg: Config, sch, c, table, F, S, self_extra):
    """table [N, F] f32 -> [128, B*F] e4m3 slot stream for core c.
    Slot value = S * dsc[slot] * table[sid[slot]]; self_extra [SHARD_PAD, F]
    (S * per-node bias term) is added onto the self-loop slots."""
    sid, dsc = sch["sid"][c], sch["dsc"][c]
    B = sid.shape[0] // cfg.PW
    m = np.zeros((sid.shape[0], F), dtype=np.float32)
    valid = sid >= 0
    m[valid] = table[sid[valid]] * (dsc[valid] * S)[:, None]
    if self_extra is not None:
        m[sch["self_slots"]] += self_extra
    m = m.astype(E4M3)
    # slot s = b*128 + p  ->  [p, b, f]
    m = np.ascontiguousarray(m.reshape(B, cfg.PW, F).transpose(1, 0, 2))
    return m.reshape(cfg.PW, B * F)


def unpack_feature_major(cfg: Config, tab, rows, node_of):
    """rows [F, SHARD_PAD] device output -> scatter into full [N, F]
    table by orig node id (cores own disjoint node sets)."""
    a = np.asarray(rows, dtype=np.float32).T       # [SHARD_PAD, F]
    valid = node_of >= 0
    tab[node_of[valid]] = a[valid]


def build_dense(cfg: Config):
    """NEFF-0: h1 = xT.T @ W1 per shard (xT pre-scaled by dis on host)."""
    import concourse.bacc as bacc
    import concourse.mybir as mybir
    from concourse import tile

    dt = mybir.dt
    AF = mybir.ActivationFunctionType
    NPW, PW, F0, F1 = cfg.NPW, cfg.PW, cfg.F0, cfg.F1

    nc = bacc.Bacc("TRN2", target_bir_lowering=False, debug=False,
                   num_devices=cfg.NC)
    nc.move_matmul_waits_to_ldweights = lambda: None
    xT = nc.dram_tensor("xT", [F0, cfg.SHARD_PAD], dt.bfloat16,
                        kind="ExternalInput")
    W1t = nc.dram_tensor("W1t", [F0, F1], dt.bfloat16, kind="ExternalInput")
    h1 = nc.dram_tensor("h1", [F1, cfg.SHARD_PAD], dt.bfloat16,
                        kind="ExternalOutput")

    GW = 4    # windows per matmul (512 moving cols)
    with tile.TileContext(nc) as tc:
        with (
            tc.tile_pool(name="const", bufs=1) as constp,
            tc.tile_pool(name="ps", bufs=4, space="PSUM") as psp,
        ):
            w1s = constp.tile([F0, F1], dt.bfloat16)
            nc.gpsimd.dma_start(w1s[:, :], W1t[:, :])
            xt = constp.tile([128, cfg.SHARD_PAD], dt.bfloat16)
            # 3 big chunks: small leader (sync), middle (gpsimd SWDGE),
            # tail (sync) — consecutive DMAs on one queue have a ~5.5us
            # gap, so few big chunks on two queues deliver fastest
            mid = (cfg.SHARD_PAD // 2 // PW) * PW
            nc.sync.dma_start(xt[:, 0:1024], xT[:, 0:1024])
            nc.scalar.dma_start(xt[:, 1024:mid], xT[:, 1024:mid])
            nc.sync.dma_start(xt[:, mid:], xT[:, mid:])
            h_full = constp.tile([F1, cfg.SHARD_PAD], dt.bfloat16)
            wrote = 0
            for g0 in range(0, NPW, GW):
                gn = min(GW, NPW - g0)
                ps = psp.tile([F1, GW * PW], dt.float32, tag="ps")
                nc.tensor.matmul(out=ps[:, :gn * PW], lhsT=w1s[:, :],
                                 rhs=xt[:, g0 * PW:(g0 + gn) * PW],
                                 start=True, stop=True)
                if (g0 // GW) % 2 == 0:
                    nc.scalar.activation(
                        h_full[:, g0 * PW:(g0 + gn) * PW],
                        ps[:, :gn * PW], AF.Copy)
                else:
                    nc.vector.tensor_copy(
                        h_full[:, g0 * PW:(g0 + gn) * PW],
                        ps[:, :gn * PW])
                done = g0 + gn
                if done - wrote >= 48 or done == NPW:
                    nc.sync.dma_start(h1[:, wrote * PW:done * PW],
                                      h_full[:, wrote * PW:done * PW])
                    wrote = done
    _dedup_ldweights(nc)
    nc.compile()
    return nc


def build_edge(cfg: Config, sch, layer):
    """NEFF-A (layer=1): fp8 DoubleRow aggregation + relu -> z';
         XBAR pair transposes; T2 = z'^T.T @ W2'   -> h2 [F2, nodes] bf16
       NEFF-B (layer=2): fp8 DoubleRow aggregation + (1/S2) copy
                                                    -> out [128, NPW*F2]
    """
    import concourse.bacc as bacc
    import concourse.mybir as mybir
    from concourse import tile
    from concourse.masks import make_identity

    dt = mybir.dt
    AF = mybir.ActivationFunctionType
    DR = mybir.MatmulPerfMode.DoubleRow
    NPW, PW = cfg.NPW, cfg.PW
    F1, F2 = cfg.F1, cfg.F2
    FM = F1 if layer == 1 else F2
    groups, B, maxKG = sch["groups"], sch["B"], sch["maxKG"]
    TB = cfg.TB1 if layer == 1 else cfg.TB2

    nc = bacc.Bacc("TRN2", target_bir_lowering=False, debug=False,
                   num_devices=cfg.NC)
    nc.move_matmul_waits_to_ldweights = lambda: None

    msgs = nc.dram_tensor("msgs", [128, B * FM], dt.float8e4,
                          kind="ExternalInput")
    id2 = nc.dram_tensor("id2", [128, 2 * 128], dt.float8e4,
                         kind="ExternalInput")   # stacked identity
    if layer == 1:
        W2t = nc.dram_tensor("W2t", [128, F2], dt.bfloat16,
                             kind="ExternalInput")  # W2/S1 at rows 0:48,64:112
        h2 = nc.dram_tensor("h2", [F2, cfg.SHARD_PAD], dt.bfloat16,
                            kind="ExternalOutput")
    else:
        inv = nc.dram_tensor("inv", [PW, 1], dt.float32,
                             kind="ExternalInput")  # 1/S2
        out = nc.dram_tensor("out", [128, NPW * F2], dt.bfloat16,
                             kind="ExternalOutput")

    dve_groups = set()    # DVE tensor_reduce measured 3.5ns/elem: unusable
    with tile.TileContext(nc) as tc:
        with (
            tc.tile_pool(name="const", bufs=1) as constp,
            tc.tile_pool(name="zr", bufs=2) as zrp,
            tc.tile_pool(name="ps", bufs=3, space="PSUM") as psp,
            tc.tile_pool(name="psw", bufs=2, space="PSUM") as pswp,
        ):
            # The whole fp8 stream fits in SBUF. Fetch it upfront into ONE
            # resident tile via a few BIG chunk DMAs: consecutive DMAs on
            # one queue have a ~5.5us dead gap between transfers, so use
            # few chunks (>=2MB after the first) alternating between the
            # sync (HWDGE) and gpsimd (SWDGE) queues; scalar stays clean
            # for the relu/XBAR chain.
            stream = constp.tile([128, B * FM], dt.float8e4)
            bounds = [g["start"] * FM for g in groups] + [B * FM]
            chunks = []
            lo = 0
            for gi in range(1, len(groups) + 1):
                target = 4000 if not chunks else 18000
                if bounds[gi] - lo >= target or gi == len(groups):
                    chunks.append((lo, bounds[gi]))
                    lo = bounds[gi]
            # layer 1: scalar is saturated by the relu/XBAR chain, so pair
            # sync with gpsimd (SWDGE drain is off the critical path there).
            # layer 2: scalar is nearly idle; SWDGE's slow end-drain was on
            # the critical path, so pair sync with scalar (chunks issued
            # before any waiting instruction joins the scalar queue).
            eng2 = nc.gpsimd if layer == 1 else nc.scalar
            for ci, (a, b) in enumerate(chunks):
                eng = (nc.sync, eng2)[ci % 2]
                eng.dma_start(stream[:, a:b], msgs[:, a:b])

            ident2 = constp.tile([128, 2, 128], dt.float8e4)
            if layer == 1:
                nc.scalar.dma_start(ident2[:, :, :], id2[:, :])
                w2s = constp.tile([128, F2], dt.bfloat16)
                nc.scalar.dma_start(w2s[:, :], W2t[:, :])
                z_all = constp.tile([128, NPW * 64], dt.bfloat16)
                zT = constp.tile([128, (NPW // 2) * 128], dt.bfloat16)
                t2_all = constp.tile([F2, cfg.SHARD_PAD], dt.bfloat16)
                t2q = t2_all[:, :].rearrange("a (j rq) -> a j rq", rq=256)
            else:
                nc.gpsimd.dma_start(ident2[:, :, :], id2[:, :])
                invs = constp.tile([PW, 1], dt.float32)
                nc.gpsimd.dma_start(invs[:, :], inv[:, :])
                o_full = constp.tile([128, NPW * F2], dt.bfloat16)

            def emit_w2(p_lo, p_hi):
                """T2 = z'^T.T @ W2' over transposed pairs [p_lo, p_hi):
                evens then odds series, 512-col chunks, PSUM->t2_all on
                vector, then flush exactly these windows to DRAM."""
                ccols = (p_hi - p_lo) * 128
                for half, r0 in ((0, 0), (1, 64)):
                    for c0 in range(0, ccols, 512):
                        cw = min(512, ccols - c0)
                        ps2 = pswp.tile([F2, 512], dt.float32, tag="ps2")
                        nc.tensor.matmul(
                            out=ps2[:, :cw],
                            lhsT=w2s[r0:r0 + F1, :],
                            rhs=zT[r0:r0 + F1,
                                   p_lo * 128 + c0:p_lo * 128 + c0 + cw],
                            start=True, stop=True)
                        npc = cw // 128
                        j0 = p_lo + c0 // 128
                        dv = t2q[:, j0:j0 + npc,
                                 half * 128:half * 128 + 128]
                        sv = ps2[:, :cw].rearrange("a (j q) -> a j q", q=128)
                        nc.vector.tensor_copy(dv, sv)
                nc.gpsimd.dma_start(h2[:, p_lo * 256:p_hi * 256],
                                    t2_all[:, p_lo * 256:p_hi * 256])

            wrote = [NPW]   # layer-2 flush high-water (window ids descend)
            top = [NPW // 2]      # highest un-W2'd pair bound
            p0s = []              # per-group lowest pair index history
            for gi, g in enumerate(groups):
                gn, KG, w0 = g["gn"], g["KG"], g["w0"]
                cols = gn * FM
                s0 = g["start"] * FM
                gv = stream[:, s0:s0 + KG * cols].rearrange(
                    "p (k c) -> p k c", c=cols)
                if gi in dve_groups:
                    # DVE strided-k reduce keeps the PE free
                    acc = zrp.tile([128, TB * FM], dt.float32, tag="zr")
                    nc.vector.tensor_reduce(
                        acc[:, :cols], gv.rearrange("p k c -> p c k"),
                        mybir.AxisListType.X, mybir.AluOpType.add)
                else:
                    acc = psp.tile([128, TB * FM], dt.float32, tag="ps")
                    for k in range(0, KG, 2):
                        nc.tensor.matmul(out=acc[:, :cols],
                                         lhsT=ident2[:, :, :],
                                         rhs=gv[:, k:k + 2, :],
                                         start=(k == 0), stop=(k == KG - 2),
                                         perf_mode=DR)
                if layer == 1:
                    # relu -> z' into 64-aligned window slots (one ACT)
                    zv = z_all[:, w0 * 64:(w0 + gn) * 64].rearrange(
                        "p (w f) -> p w f", f=64)[:, :, 0:F1]
                    pv = acc[:, :cols].rearrange("p (w f) -> p w f", f=F1)
                    nc.scalar.activation(zv, pv, AF.Relu)
                    # XBAR pair transposes: [128, gn*64] -> gn/2 slabs
                    p0 = w0 // 2
                    npair = gn // 2
                    tv = zT[:, p0 * 128:(p0 + npair) * 128].rearrange(
                        "p (j q) -> p j q", q=128)
                    nc.scalar.dma_start(tv,
                                        z_all[:, w0 * 64:(w0 + gn) * 64],
                                        transpose=True)
                    # W2 lags TWO groups so its XBAR wait never stalls the
                    # in-order PE queue
                    p0s.append(p0)
                    allowed = p0s[-3] if len(p0s) >= 3 else None
                    if allowed is not None and top[0] - allowed >= 10:
                        emit_w2(allowed, top[0])
                        top[0] = allowed
                else:
                    ov = o_full[:, w0 * F2:(w0 + gn) * F2]
                    if gi in dve_groups:
                        nc.vector.tensor_scalar_mul(ov, acc[:, :cols],
                                                    invs[:, :])
                    elif gi % 2 == 0:
                        nc.vector.tensor_scalar_mul(ov, acc[:, :cols],
                                                    invs[:, :])
                    else:
                        nc.scalar.activation(ov, acc[:, :cols], AF.Copy,
                                             scale=invs[:, :])
                    hi = wrote[0]
                    if hi - w0 >= 24 or gi == len(groups) - 1:
                        nc.scalar.dma_start(out[:, w0 * F2:hi * F2],
                                            o_full[:, w0 * F2:hi * F2])
                        wrote[0] = w0
            if layer == 1 and top[0] > p0s[-1]:
                emit_w2(p0s[-1], top[0])
    _dedup_ldweights(nc)
    nc.compile()
    return nc


EXEC_LOG = []  # (exec_time_ns, trace_path) per launch when BASS_TRACE=1


def run_spmd(cfg: Config, nc, in_maps):
    from concourse.bass_utils import run_bass_kernel_spmd
    res = run_bass_kernel_spmd(nc, in_maps=in_maps,
                               core_ids=list(range(cfg.NC)))
    trace_path = None
    if res.instructions_and_trace is not None:
        trace_path = res.instructions_and_trace[1]
    EXEC_LOG.append((res.exec_time_ns, trace_path))
    return res.results


def kernel(x, edge_index, W1, b1, W2, b2):
    cfg = CFG
    N, NC, PW, NPW = cfg.N, cfg.NC, cfg.PW, cfg.NPW
    meta = preprocess(cfg, edge_index)
    dis, dis2 = meta["dis"], meta["dis2"]
    sqd = 1.0 / dis

    x = np.asarray(x, dtype=np.float32)
    xs = x * dis[:, None]
    b1 = np.asarray(b1, dtype=np.float32).reshape(1, cfg.F1)
    b2 = np.asarray(b2, dtype=np.float32).reshape(1, cfg.F2)

    in0 = []
    for c in range(NC):
        nod = meta["node_of"][c]
        valid = nod >= 0
        xc = np.zeros((cfg.SHARD_PAD, cfg.F0), dtype=np.float32)
        xc[valid] = xs[nod[valid]]
        xT = np.ascontiguousarray(xc.T).astype(BF16)
        in0.append({"xT": xT, "W1t": _to_bf16(W1)})

    nc0 = build_dense(cfg)
    res0 = run_spmd(cfg, nc0, in0)
    T1 = np.zeros((N, cfg.F1), dtype=np.float32)
    for c in range(NC):
        unpack_feature_major(cfg, T1, res0[c]["h1"], meta["node_of"][c])

    # S1: max |stream value| = max(dis2_v*|T1[u]|, dis2_v*|T1[v]+sqd_v*b1|)
    sch1 = meta["sched1"]
    rmax1 = np.abs(T1).max(axis=1)
    selfv1 = T1 + sqd[:, None] * b1
    mx = 0.0
    for c in range(NC):
        sid, dsc = sch1["sid"][c], sch1["dsc"][c]
        v = sid >= 0
        m = float((np.abs(rmax1[sid[v]]) * dsc[v]).max())
        mx = max(mx, m)
    mx = max(mx, float((dis2[:, None] * np.abs(selfv1)).max()))
    S1 = pow2_gain(mx)

    ncA = build_edge(cfg, sch1, layer=1)
    eye = np.eye(128, dtype=np.float32)
    id2 = np.concatenate([eye, eye], axis=1).astype(E4M3)
    w2dup = np.zeros((128, cfg.F2), dtype=np.float32)
    w2v = np.asarray(W2, dtype=np.float32) / S1
    w2dup[0:cfg.F1] = w2v
    w2dup[64:64 + cfg.F1] = w2v
    inA = []
    for c in range(NC):
        nod = meta["node_of"][c]
        valid = nod >= 0
        ext = np.zeros((cfg.SHARD_PAD, cfg.F1), dtype=np.float32)
        ext[valid] = (S1 * dis[nod[valid], None]) * b1
        inA.append({"msgs": gather_stream(cfg, sch1, c, T1, cfg.F1, S1, ext),
                    "W2t": _to_bf16(w2dup), "id2": id2})
    resA = run_spmd(cfg, ncA, inA)
    T2 = np.zeros((N, cfg.F2), dtype=np.float32)
    for c in range(NC):
        unpack_feature_major(cfg, T2, resA[c]["h2"], meta["node_of"][c])

    sch2 = meta["sched2"]
    rmax2 = np.abs(T2).max(axis=1)
    selfv2 = dis[:, None] * T2 + b2
    mx = 0.0
    for c in range(NC):
        sid, dsc = sch2["sid"][c], sch2["dsc"][c]
        v = sid >= 0
        m = float((np.abs(rmax2[sid[v]]) * dsc[v]).max())
        mx = max(mx, m)
    mx = max(mx, float(np.abs(selfv2).max()))
    S2 = pow2_gain(mx)

    ncB = build_edge(cfg, sch2, layer=2)
    inB = []
    invv = np.full((PW, 1), 1.0 / S2, dtype=np.float32)
    for c in range(NC):
        nod = meta["node_of"][c]
        valid = nod >= 0
        # self slot extra: dsc already carries dis_v; slot = S2*dis_v*T2[v]
        # + S2*b2  (dis*sqd = 1)
        ext = np.zeros((cfg.SHARD_PAD, cfg.F2), dtype=np.float32)
        ext[valid] = S2 * b2
        inB.append({"msgs": gather_stream(cfg, sch2, c, T2, cfg.F2, S2, ext),
                    "inv": invv, "id2": id2})
    resB = run_spmd(cfg, ncB, inB)

    out = np.zeros((N, cfg.F2), dtype=np.float32)
    for c in range(NC):
        rows = np.asarray(resB[c]["out"]).astype(np.float32)
        a = rows.reshape(cfg.PW, NPW, cfg.F2).transpose(1, 0, 2).reshape(
            -1, cfg.F2)
        nod = meta["node_of"][c]
        valid = nod >= 0
        out[nod[valid]] = a[valid]
    return out
